# revision 21
# baseline (speedup 1.0000x reference)
"""Trainium2 Bass kernel for nn_ChemROAR (single transformer block, B=8).

Sharding: data-parallel over batch — core b computes batch element b.
No collectives. Matmuls run in float32r (rounded fp32, 1 cycle/row on PE
vs 4 for plain fp32).

Self-contained: only imports from /opt/trn_rl_repo (present on the target
machine image); no sibling files.
"""
import sys
import types

sys.path.insert(0, "/opt/trn_rl_repo")

import numpy as np

import concourse.bass as bass
import concourse.mybir as mybir
import concourse.tile as tile
import concourse.tile_utils as tile_utils
from concourse.vector_clock import ScopedClock

F32 = mybir.dt.float32
F32R = mybir.dt.float32r
BF16 = mybir.dt.bfloat16
I32 = mybir.dt.int32
AF = mybir.ActivationFunctionType
ALU = mybir.AluOpType

P = 128
B, T, D, H, DFF, NTYPE = 8, 1024, 512, 8, 1024, 341
HD = D // H          # 64
DPR = 32             # rotary dims per head
TT = T // P          # 8 token tiles
DK = D // P          # 4 d chunks
EPS = 1e-5
THETA = 10000.0
TWO_PI = 6.283185307179586
INV_2PI = 1.0 / TWO_PI
MAGIC = 12582912.0   # 1.5 * 2**23 — round-to-nearest magic for fp32
NH = HD + 1          # v columns + ones column (softmax denominator)
NCH = 2              # Tq chunks per head
CW = T // NCH        # 512

# SBUF cap: tile_utils default (192 KiB/partition) is stale; cayman has
# 208 KiB usable. Stay a bit under.
tile_utils.max_sbuf_usage = 206 * 1024

# ---------------------------------------------------------------------------
# Patch 1: the public walrus accepts only ONE attached sync-wait per
# instruction. Split excess waits onto standalone NoOps placed before the
# instruction (and split the kernel-tail drain into a chain of drains).
# ---------------------------------------------------------------------------
_MAXW = 1


def _install_tile_patch():
    if getattr(tile.TileContext, "_chemroar_patched", False):
        return
    orig_commit = tile.TileContext._commit_instruction

    def _commit_instruction(self, inst, lazy_reg_writes=True):
        si = getattr(inst, "sync_info", None)
        if si is not None and si.on_wait:
            waits = list(si.on_wait)
            if len(waits) > _MAXW:
                keep = waits[:_MAXW]
                excess = waits[_MAXW:]
                for i in range(0, len(excess), _MAXW):
                    nop = mybir.InstNoOp(
                        name=self.nc.get_next_instruction_name(),
                        ins=[],
                        outs=[],
                        sync_info=mybir.SyncInfo(
                            on_wait=excess[i : i + _MAXW], on_update=[]
                        ),
                        bass_nofuse=True,
                        engine=inst.engine,
                    )
                    self._add_instruction(nop)
                inst.sync_info = mybir.SyncInfo(
                    on_wait=keep, on_update=list(si.on_update)
                )
        return orig_commit(self, inst, lazy_reg_writes=lazy_reg_writes)

    def _drain_and_barrier(self, tick_clock, wait_clock):
        drain_inst = self.nc.sync.drain()
        wait_clock.add_sem_waits(
            drain_inst.ins, ScopedClock({None: tick_clock.global_clock})
        )
        mi = drain_inst.ins
        si = mi.sync_info
        if si is not None and si.on_wait and len(si.on_wait) > _MAXW:
            waits = list(si.on_wait)
            mi.sync_info = mybir.SyncInfo(
                on_wait=waits[:_MAXW], on_update=list(si.on_update)
            )
            for i in range(_MAXW, len(waits), _MAXW):
                d2 = self.nc.sync.drain()
                d2.ins.sync_info = mybir.SyncInfo(
                    on_wait=waits[i : i + _MAXW], on_update=[]
                )
        self.nc.all_engine_barrier()
        assert self.sems is not None
        popped = self.nc._tile_sem_poison_stack.pop()
        assert popped is self._sem_poison
        self.nc.clear_and_free_semaphores(list(self.sems.allocated().values()))
        self.nc.all_engine_barrier()

    tile.TileContext._commit_instruction = _commit_instruction
    tile.TileContext._drain_and_barrier = _drain_and_barrier
    tile.TileContext._chemroar_patched = True


_install_tile_patch()


# ---------------------------------------------------------------------------
# Patch 2: NTFF profile hook (the stripped antenv lacks axon_hooks).
# ---------------------------------------------------------------------------
def _install_hookfix():
    name = "antenv.axon_hooks"
    if name in sys.modules:
        return
    try:
        from trn_agent_boot.trn_boot import _ntff_profile_via_ctypes

        hook = _ntff_profile_via_ctypes("/opt/axon/libaxon_pjrt.so")
    except Exception:
        hook = None
    mod = types.ModuleType(name)
    mod._hook = hook
    mod.set_axon_ntff_profile_hook = lambda h: setattr(mod, "_hook", h)
    mod.get_axon_ntff_profile_hook = lambda: mod._hook
    sys.modules[name] = mod
    try:
        import antenv

        antenv.axon_hooks = mod
    except Exception:
        pass


_install_hookfix()


def _ap_with(a, offset_delta, ap_list):
    import dataclasses

    return dataclasses.replace(a, offset=a.offset + offset_delta, ap=ap_list)


def build_nc(trivial_ln1, trivial_ln2, trivial_b1, trivial_b2):
    nc = bass.Bass("TRN2", target_bir_lowering=False, debug=False)

    xv_d = nc.declare_dram_parameter("xv", [T, D], F32, isOutput=False)
    wa_d = nc.declare_dram_parameter("wa", [D, 3 * D], F32, isOutput=False)
    w1_d = nc.declare_dram_parameter("w1", [D, 2 * DFF], F32, isOutput=False)
    w2_d = nc.declare_dram_parameter("w2", [DFF, D], F32, isOutput=False)
    teq_d = nc.declare_dram_parameter("teq", [NTYPE, D], F32, isOutput=False)
    tek_d = nc.declare_dram_parameter("tek", [NTYPE, D], F32, isOutput=False)
    xtq_d = nc.declare_dram_parameter("xtq", [T], I32, isOutput=False)
    xtk_d = nc.declare_dram_parameter("xtk", [T], I32, isOutput=False)
    posq_d = nc.declare_dram_parameter("posq", [T], F32, isOutput=False)
    posk_d = nc.declare_dram_parameter("posk", [T], F32, isOutput=False)
    ident_d = nc.declare_dram_parameter("ident", [P, P], F32, isOutput=False)
    invf_d = nc.declare_dram_parameter("invf", [P, 16], F32, isOutput=False)
    g1_d = nc.declare_dram_parameter("g1", [D], F32, isOutput=False)
    b1ln_d = nc.declare_dram_parameter("b1ln", [D], F32, isOutput=False)
    g2_d = nc.declare_dram_parameter("g2", [D], F32, isOutput=False)
    b2ln_d = nc.declare_dram_parameter("b2ln", [D], F32, isOutput=False)
    bf1_d = nc.declare_dram_parameter("bf1", [2 * DFF], F32, isOutput=False)
    bf2_d = nc.declare_dram_parameter("bf2", [D], F32, isOutput=False)
    out_d = nc.declare_dram_parameter("out", [T, D], F32, isOutput=True)

    with tile.TileContext(nc) as tc:
        wpool = tc.alloc_tile_pool(name="wpool", bufs=1)
        work = tc.alloc_tile_pool(name="work", bufs=1)
        spool = tc.alloc_tile_pool(name="spool", bufs=2)
        psum = tc.alloc_tile_pool(name="psum", bufs=4, space="PSUM")
        psum_tr = tc.alloc_tile_pool(name="psum_tr", bufs=2, space="PSUM")
        psum_o = tc.alloc_tile_pool(name="psum_o", bufs=2, space="PSUM")

        # ---------------- small constants ----------------
        ident = wpool.tile([P, P], F32)
        nc.sync.dma_start(ident[:], ident_d.ap())
        identr = wpool.tile([P, P], F32R)
        nc.gpsimd.tensor_copy(identr[:], ident[:])

        invf = wpool.tile([P, 16], F32)
        nc.sync.dma_start(invf[:], invf_d.ap())

        posq_sb = wpool.tile([P, TT], F32)
        nc.sync.dma_start(posq_sb[:], posq_d.ap().rearrange("(a p) -> p a", p=P))
        posk_sb = wpool.tile([P, TT], F32)
        nc.sync.dma_start(posk_sb[:], posk_d.ap().rearrange("(a p) -> p a", p=P))

        if not trivial_b1:
            bf1_sb = wpool.tile([P, 2 * DFF // P], F32)
            nc.sync.dma_start(bf1_sb[:], bf1_d.ap().rearrange("(o p) -> p o", p=P))
        if not trivial_b2:
            bf2_sb = wpool.tile([P, DK], F32)
            nc.sync.dma_start(bf2_sb[:], bf2_d.ap().rearrange("(o p) -> p o", p=P))

        def load_weight_f32r(dram_ap, ko, n, tag):
            """DMA [ko*P, n] DRAM weight, cast to F32R via a scratch ring."""
            wr = work.tile([P, ko, n], F32R, tag=tag)
            src = dram_ap.rearrange("(ko ki) n -> ki ko n", ki=P)
            CHW = 256
            for k in range(ko):
                for c0 in range(0, n, CHW):
                    w = min(CHW, n - c0)
                    sc = spool.tile([P, CHW], F32, tag="wscratch", bufs=3)
                    nc.sync.dma_start(sc[:, :w], src[:, k, c0 : c0 + w])
                    nc.gpsimd.tensor_copy(wr[:, k, c0 : c0 + w], sc[:, :w])
            return wr

        # gamma/beta partition-broadcast tiles via K=1 matmul
        def bcast_row(src_dram, n, tag):
            row = wpool.tile([1, n], F32, tag=f"bcrow_{tag}")
            nc.sync.dma_start(row[:], src_dram.ap().rearrange("(o n) -> o n", o=1))
            rowr = wpool.tile([1, n], F32R, tag=f"bcrowr_{tag}")
            nc.vector.tensor_copy(rowr[:], row[:])
            onesc = wpool.tile([1, P], F32R, tag="bc_ones")
            nc.vector.memset(onesc[:], 1.0)
            out_t = wpool.tile([P, n], F32, tag=f"bcout_{tag}")
            for c0 in range(0, n, 512):
                w = min(512, n - c0)
                pt = psum_o.tile([P, CW], F32, tag="o_ps")
                nc.tensor.matmul(
                    pt[:, :w], lhsT=onesc[:], rhs=rowr[:, c0 : c0 + w],
                    start=True, stop=True,
                )
                nc.scalar.copy(out_t[:, c0 : c0 + w], pt[:, :w])
            return out_t

        g1_bc = b1_bc = g2_bc = b2_bc = None
        if not trivial_ln1:
            g1_bc = bcast_row(g1_d, D, "g1")
            b1_bc = bcast_row(b1ln_d, D, "b1")
        if not trivial_ln2:
            g2_bc = bcast_row(g2_d, D, "g2")
            b2_bc = bcast_row(b2ln_d, D, "b2")

        # attention weights now; FFN weights later (lifetime-shared tags)
        war = load_weight_f32r(wa_d.ap(), DK, 3 * D, tag="w_big")

        # ---------------- load x ----------------
        xs = work.tile([P, TT, D], F32, tag="xs_h2T")
        for ti in range(TT):
            nc.sync.dma_start(xs[:, ti, :], xv_d.ap()[ti * P : (ti + 1) * P, :])

        # ---------------- helpers ----------------
        def layernorm_tile(x_ap, out_ap, g_bc, b_bc, trivial):
            m = spool.tile([P, 1], F32, tag="ln_m")
            nc.vector.reduce_sum(m[:], x_ap, axis=mybir.AxisListType.X)
            nc.vector.tensor_scalar_mul(m[:], m[:], 1.0 / D)
            sq = spool.tile([P, 1], F32, tag="ln_sq")
            # out_ap doubles as junk output for the squared pass
            nc.scalar.activation(out_ap, x_ap, AF.Square, accum_out=sq[:])
            mm2 = spool.tile([P, 1], F32, tag="ln_mm2")
            nc.vector.tensor_tensor(mm2[:], m[:], m[:], ALU.mult)
            s = spool.tile([P, 1], F32, tag="ln_s")
            nc.vector.tensor_scalar(s[:], sq[:], 1.0 / D, EPS, ALU.mult, ALU.add)
            nc.vector.tensor_tensor(s[:], s[:], mm2[:], ALU.subtract)
            nc.vector.reciprocal(s[:], s[:])
            nc.scalar.sqrt(s[:], s[:])
            if trivial:
                nc.vector.tensor_scalar(out_ap, x_ap, m[:], s[:],
                                        ALU.subtract, ALU.mult)
            else:
                tmp = spool.tile([P, D], F32, tag="ring_eq_sig")
                nc.vector.tensor_scalar(tmp[:], x_ap, m[:], s[:],
                                        ALU.subtract, ALU.mult)
                nc.vector.tensor_tensor(tmp[:], tmp[:], g_bc[:], ALU.mult)
                nc.vector.tensor_tensor(out_ap, tmp[:], b_bc[:], ALU.add)

        _tr_flip = [0]

        def transpose_128(src_ap, dst_ap):
            pt = psum_tr.tile([P, P], F32R, tag="tr_ps")
            nc.tensor.transpose(pt[:], src_ap, identr[:])
            _tr_flip[0] ^= 1
            if _tr_flip[0]:
                nc.vector.tensor_copy(dst_ap, pt[:])
            else:
                nc.scalar.copy(dst_ap, pt[:])

        # ---------------- LN1 + transpose h (per-tile ring) --------------
        hT = work.tile([P, DK, T], F32R, tag="hT_qT")
        for ti in range(TT):
            h_t = spool.tile([P, D], F32R, tag="h_ring")
            layernorm_tile(xs[:, ti, :], h_t[:], g1_bc, b1_bc, trivial_ln1)
            for j in range(DK):
                transpose_128(
                    h_t[:, j * P : (j + 1) * P],
                    hT[:, j, ti * P : (ti + 1) * P],
                )

        # ---------------- qkv + emb + rope + transpose --------------------
        # NOTE: qT shares the hT tag slot, so allocate it only after hT's
        # last use. We therefore first compute q_sb/k_sb/vext fully.
        q_sb = work.tile([P, TT, D], F32R, tag="q_sb")
        k_sb = work.tile([P, TT, D], F32R, tag="k_sb")
        vext = work.tile([P, TT, H, NH], BF16, tag="vext_w2")
        onesf = wpool.tile([P, H], F32, tag="onesf")
        nc.gpsimd.memset(onesf[:], 1.0)
        for ti in range(TT):
            nc.gpsimd.tensor_copy(
                vext[:, ti, :, HD : HD + 1],
                onesf[:].rearrange("p (h o) -> p h o", o=1),
            )

        for ti in range(TT):
            # emb gathers for this tile
            offq = spool.tile([P, 1], I32, tag="offq")
            nc.sync.dma_start(
                offq[:], xtq_d.ap()[ti * P : (ti + 1) * P].rearrange("(p o) -> p o", o=1)
            )
            eq = spool.tile([P, D], F32, tag="ring_eq_sig")
            nc.gpsimd.indirect_dma_start(
                out=eq[:],
                out_offset=None,
                in_=teq_d.ap(),
                in_offset=bass.IndirectOffsetOnAxis(ap=offq[:], axis=0),
            )
            offk = spool.tile([P, 1], I32, tag="offk")
            nc.sync.dma_start(
                offk[:], xtk_d.ap()[ti * P : (ti + 1) * P].rearrange("(p o) -> p o", o=1)
            )
            ek = spool.tile([P, D], F32, tag="ek_ring")
            nc.gpsimd.indirect_dma_start(
                out=ek[:],
                out_offset=None,
                in_=tek_d.ap(),
                in_offset=bass.IndirectOffsetOnAxis(ap=offk[:], axis=0),
            )
            pts = {}
            for which in ("q", "k", "v"):
                pts[which] = psum.tile([P, CW], F32, tag="mm_ps", name=f"qkv_{which}")
            for kk in range(DK):
                for wi, (which, base) in enumerate(
                    (("q", 0), ("k", D), ("v", 2 * D))
                ):
                    nc.tensor.matmul(
                        pts[which][:, :D],
                        lhsT=hT[:, kk, ti * P : (ti + 1) * P],
                        rhs=war[:, kk, base : base + D],
                        start=(kk == 0),
                        stop=(kk == DK - 1),
                    )
            nc.vector.tensor_tensor(q_sb[:, ti, :], pts["q"][:, :D], eq[:], ALU.add)
            nc.vector.tensor_tensor(k_sb[:, ti, :], pts["k"][:, :D], ek[:], ALU.add)
            nc.vector.tensor_copy(
                vext[:, ti, :, 0:HD],
                pts["v"][:, :D].rearrange("p (h x) -> p h x", h=H),
            )

        # ---------------- RoPE (token-major, in place) --------------------
        def rope_tile(dst, ti, pos_sb):
            fr = spool.tile([P, 16], F32, tag="rp_fr")
            nc.vector.tensor_scalar_mul(fr[:], invf[:], pos_sb[:, ti : ti + 1])

            def lut_arg(tag, quarter):
                y = spool.tile([P, 16], F32, tag=f"rp_y{tag}")
                nc.vector.tensor_scalar(
                    y[:], fr[:], INV_2PI, 0.25 if quarter else 0.0,
                    ALU.mult, ALU.add,
                )
                kk = spool.tile([P, 16], F32, tag=f"rp_k{tag}")
                nc.vector.tensor_scalar(
                    kk[:], y[:], MAGIC, MAGIC, ALU.add, ALU.subtract
                )
                ang = spool.tile([P, 16], F32, tag=f"rp_a{tag}")
                nc.vector.scalar_tensor_tensor(
                    ang[:], kk[:], -TWO_PI, fr[:], ALU.mult, ALU.add
                )
                if quarter:
                    nc.vector.tensor_scalar_add(ang[:], ang[:], np.pi / 2)
                sc = spool.tile([P, 16], F32, tag=f"rp_s{tag}")
                nc.scalar.activation(sc[:], ang[:], AF.Sin)
                return sc

            sin16 = lut_arg("s", False)
            cos16 = lut_arg("c", True)
            cos32 = spool.tile([P, 32], F32, tag="rp_cos32")
            c32v = cos32[:].rearrange("p (u v) -> p u v", v=2)
            nc.vector.tensor_copy(c32v[:, :, 0], cos16[:])
            nc.vector.tensor_copy(c32v[:, :, 1], cos16[:])
            sin32 = spool.tile([P, 32], F32, tag="rp_sin32")
            s32v = sin32[:].rearrange("p (u v) -> p u v", v=2)
            nc.scalar.mul(s32v[:, :, 0], sin16[:], -1.0)
            nc.vector.tensor_copy(s32v[:, :, 1], sin16[:])

            rot = (
                dst[:, ti, :]
                .rearrange("p (h x) -> p h x", h=H)[:, :, 0:DPR]
                .rearrange("p h (u v) -> p h u v", v=2)
            )
            shuf = _ap_with(rot, 1, [rot.ap[0], rot.ap[1], rot.ap[2], [-1, 2]])
            sin_b = (
                sin32[:].rearrange("p (u v) -> p u v", v=2)
                .unsqueeze(1)
                .broadcast_to((P, H, 16, 2))
            )
            cos_b = (
                cos32[:].rearrange("p (u v) -> p u v", v=2)
                .unsqueeze(1)
                .broadcast_to((P, H, 16, 2))
            )
            tmp = spool.tile([P, H, 16, 2], BF16, tag="rp_tmp")
            nc.vector.tensor_tensor(tmp[:], shuf, sin_b, ALU.mult)
            nc.vector.tensor_tensor(rot, rot, cos_b, ALU.mult)
            nc.vector.tensor_tensor(rot, rot, tmp[:], ALU.add)

        for ti in range(TT):
            rope_tile(q_sb, ti, posq_sb)
            rope_tile(k_sb, ti, posk_sb)

        # ---------------- transpose q, k (qT reuses hT slot) --------------
        qT = work.tile([P, DK, T], BF16, tag="hT_qT")
        kT = work.tile([P, DK, T], BF16, tag="kT_gT")
        for ti in range(TT):
            for j in range(DK):
                transpose_128(
                    q_sb[:, ti, j * P : (j + 1) * P],
                    qT[:, j, ti * P : (ti + 1) * P],
                )
                transpose_128(
                    k_sb[:, ti, j * P : (j + 1) * P],
                    kT[:, j, ti * P : (ti + 1) * P],
                )

        # ---------------- attention ----------------
        x_new = work.tile([P, TT, D], F32, tag="x_new")
        for j in range(H // 2):
            expTs = []
            for sub in range(2):
                expTs.append(
                    work.tile([P, TT, CW], BF16, tag="expT_bufs", bufs=2,
                              name=f"expT_{j}_{sub}")
                )
            oTs = []
            for sub in range(2):
                oTs.append(
                    work.tile([NH, T], F32, tag="oT", bufs=2,
                              name=f"oT_{j}_{sub}")
                )
            for c in range(NCH):
                lim = 4 * c + 4
                for ti in range(lim):
                    pss = []
                    for sub in range(2):
                        r0 = 64 * sub
                        ps = psum.tile([P, CW], F32, tag="mm_ps",
                                       name=f"sc_{j}_{sub}")
                        # heads 2j (rows 0-63) and 2j+1 (rows 64-127) run
                        # concurrently on disjoint PE row groups
                        nc.tensor.matmul(
                            ps[:],
                            lhsT=kT[r0 : r0 + HD, j, ti * P : (ti + 1) * P],
                            rhs=qT[r0 : r0 + HD, j, c * CW : (c + 1) * CW],
                            start=True,
                            stop=True,
                        )
                        pss.append(ps)
                    off = P * (ti - 4 * c)
                    for sub in range(2):
                        expT = expTs[sub]
                        ps = pss[sub]
                        if off <= -P:
                            nc.scalar.activation(
                                expT[:, ti, :], ps[:], AF.Exp, scale=0.125
                            )
                        else:
                            nc.scalar.activation(
                                expT[:, ti, off:CW], ps[:, off:CW], AF.Exp,
                                scale=0.125,
                            )
                            if off > 0:
                                nc.gpsimd.memset(expT[:, ti, 0:off], 0.0)
                            nc.gpsimd.affine_select(
                                out=expT[:, ti, off : off + P],
                                in_=expT[:, ti, off : off + P],
                                pattern=[[1, P]],
                                compare_op=ALU.is_ge,
                                fill=0.0,
                                base=0,
                                channel_multiplier=-1,
                            )
                pos = []
                for sub in range(2):
                    pos.append(psum_o.tile([P, CW], F32, tag="o_ps",
                                           name=f"po_{j}_{sub}"))
                for ti in range(lim):
                    for sub in range(2):
                        nc.tensor.matmul(
                            pos[sub][0:NH, :],
                            lhsT=vext[:, ti, 2 * j + sub, :],
                            rhs=expTs[sub][:, ti, :],
                            start=(ti == 0),
                            stop=(ti == lim - 1),
                        )
                for sub in range(2):
                    nc.vector.tensor_copy(
                        oTs[sub][:, c * CW : (c + 1) * CW], pos[sub][0:NH, :]
                    )
            for ti in range(TT):
                for sub in range(2):
                    hh = 2 * j + sub
                    pt = psum_tr.tile([P, P], F32, tag="tr_ps",
                                      name=f"tro_{j}_{sub}")
                    nc.tensor.matmul(
                        pt[:, 0:NH],
                        lhsT=oTs[sub][:, ti * P : (ti + 1) * P],
                        rhs=ident[0:NH, 0:NH],
                        is_transpose=True,
                        start=True,
                        stop=True,
                    )
                    rec = spool.tile([P, 1], F32, tag="rec")
                    nc.vector.reciprocal(rec[:], pt[:, HD : HD + 1])
                    nc.vector.scalar_tensor_tensor(
                        x_new[:, ti, hh * HD : (hh + 1) * HD],
                        pt[:, 0:HD],
                        rec[:],
                        xs[:, ti, hh * HD : (hh + 1) * HD],
                        ALU.mult,
                        ALU.add,
                    )

        # ---------------- FFN weights (reuse attention weight slots) ------
        w1r = load_weight_f32r(w1_d.ap(), DK, 2 * DFF, tag="w_big")
        w2r = load_weight_f32r(w2_d.ap(), DFF // P, D, tag="vext_w2")

        # ---------------- LN2 + transpose h2 (h2T reuses xs slot) ---------
        h2T = work.tile([P, DK, T], F32R, tag="xs_h2T")
        for ti in range(TT):
            h2_t = spool.tile([P, D], F32R, tag="h_ring")
            layernorm_tile(x_new[:, ti, :], h2_t[:], g2_bc, b2_bc, trivial_ln2)
            for j in range(DK):
                transpose_128(
                    h2_t[:, j * P : (j + 1) * P],
                    h2T[:, j, ti * P : (ti + 1) * P],
                )

        # ---------------- FFN ----------------
        gT = work.tile([P, DFF // P, T], F32R, tag="kT_gT")
        for m in range(DFF // P):
            sg = spool.tile([P, CW], F32, tag="ring_eq_sig")
            for c in range(NCH):
                pa = psum.tile([P, CW], F32, tag="mm_ps")
                pg = psum.tile([P, CW], F32, tag="mm_ps")
                for kk in range(DK):
                    nc.tensor.matmul(
                        pa[:],
                        lhsT=w1r[:, kk, m * P : (m + 1) * P],
                        rhs=h2T[:, kk, c * CW : (c + 1) * CW],
                        start=(kk == 0),
                        stop=(kk == DK - 1),
                    )
                for kk in range(DK):
                    nc.tensor.matmul(
                        pg[:],
                        lhsT=w1r[:, kk, DFF + m * P : DFF + (m + 1) * P],
                        rhs=h2T[:, kk, c * CW : (c + 1) * CW],
                        start=(kk == 0),
                        stop=(kk == DK - 1),
                    )
                cs = slice(c * CW, (c + 1) * CW)
                if trivial_b1:
                    nc.scalar.activation(sg[:], pg[:], AF.Sigmoid)
                    nc.vector.tensor_tensor(sg[:], pg[:], sg[:], ALU.mult)
                    nc.vector.tensor_tensor(gT[:, m, cs], pa[:], sg[:], ALU.mult)
                else:
                    bgap = bf1_sb[:, (DFF // P) + m : (DFF // P) + m + 1]
                    nc.scalar.activation(sg[:], pg[:], AF.Sigmoid, bias=bgap)
                    nc.vector.scalar_tensor_tensor(
                        sg[:], pg[:], bgap, sg[:], ALU.add, ALU.mult
                    )
                    nc.vector.scalar_tensor_tensor(
                        gT[:, m, cs], pa[:], bf1_sb[:, m : m + 1], sg[:],
                        ALU.add, ALU.mult,
                    )

        yT = work.tile([P, DK, T], F32R, tag="q_sb")
        for m in range(DK):
            for c in range(NCH):
                py = psum.tile([P, CW], F32, tag="mm_ps")
                for kk in range(DFF // P):
                    nc.tensor.matmul(
                        py[:],
                        lhsT=w2r[:, kk, m * P : (m + 1) * P],
                        rhs=gT[:, kk, c * CW : (c + 1) * CW],
                        start=(kk == 0),
                        stop=(kk == DFF // P - 1),
                    )
                cs = slice(c * CW, (c + 1) * CW)
                if trivial_b2:
                    nc.scalar.copy(yT[:, m, cs], py[:])
                else:
                    nc.vector.tensor_scalar_add(yT[:, m, cs], py[:], bf2_sb[:, m : m + 1])

        # ---------------- final transpose + residual + store -------------
        for ti in range(TT):
            fin = spool.tile([P, D], F32, tag="h_ring_f")
            for j in range(DK):
                pt = psum_tr.tile([P, P], F32R, tag="tr_ps")
                nc.tensor.transpose(pt[:], yT[:, j, ti * P : (ti + 1) * P], identr[:])
                nc.vector.tensor_tensor(
                    fin[:, j * P : (j + 1) * P],
                    pt[:],
                    x_new[:, ti, j * P : (j + 1) * P],
                    ALU.add,
                )
            nc.sync.dma_start(out_d.ap()[ti * P : (ti + 1) * P, :], fin[:])

        for p in (psum_o, psum_tr, psum, spool, work, wpool):
            p.release()

    return nc


_CACHE = {}


def _get_nc(key):
    if key not in _CACHE:
        _CACHE[key] = build_nc(*key)
    return _CACHE[key]


def make_in_maps(x_type, x_value, seq_order, W_attn, type_emb, ln1_g, ln1_b,
                 ln2_g, ln2_b, W1, b1, W2, b2):
    ident = np.eye(P, dtype=np.float32)
    inv_freq = 1.0 / (THETA ** (np.arange(0, DPR, 2, dtype=np.float32) / DPR))
    invf = np.tile(inv_freq[None, :], (P, 1)).astype(np.float32)
    in_maps = []
    for b in range(B):
        in_maps.append({
            "xv": np.ascontiguousarray(x_value[b], dtype=np.float32),
            "wa": np.asarray(W_attn, dtype=np.float32),
            "w1": np.asarray(W1, dtype=np.float32),
            "w2": np.asarray(W2, dtype=np.float32),
            "teq": np.ascontiguousarray(type_emb[:, :D], dtype=np.float32),
            "tek": np.ascontiguousarray(type_emb[:, D:], dtype=np.float32),
            "xtq": np.ascontiguousarray(x_type[b, :T], dtype=np.int32),
            "xtk": np.ascontiguousarray(x_type[b, 1 : T + 1], dtype=np.int32),
            "posq": np.ascontiguousarray(seq_order[b, :T], dtype=np.float32),
            "posk": np.ascontiguousarray(seq_order[b, 1 : T + 1], dtype=np.float32),
            "ident": ident,
            "invf": invf,
            "g1": np.asarray(ln1_g, dtype=np.float32),
            "b1ln": np.asarray(ln1_b, dtype=np.float32),
            "g2": np.asarray(ln2_g, dtype=np.float32),
            "b2ln": np.asarray(ln2_b, dtype=np.float32),
            "bf1": np.asarray(b1, dtype=np.float32),
            "bf2": np.asarray(b2, dtype=np.float32),
        })
    return in_maps


def triviality_key(ln1_g, ln1_b, ln2_g, ln2_b, b1, b2):
    return (
        bool(np.all(np.asarray(ln1_g) == 1.0) and np.all(np.asarray(ln1_b) == 0.0)),
        bool(np.all(np.asarray(ln2_g) == 1.0) and np.all(np.asarray(ln2_b) == 0.0)),
        bool(np.all(np.asarray(b1) == 0.0)),
        bool(np.all(np.asarray(b2) == 0.0)),
    )


def kernel(x_type, x_value, seq_order, W_attn, type_emb, ln1_g, ln1_b,
           ln2_g, ln2_b, W1, b1, W2, b2, _trace=False):
    from concourse.bass_utils import run_bass_kernel_spmd

    key = triviality_key(ln1_g, ln1_b, ln2_g, ln2_b, b1, b2)
    nc = _get_nc(key)
    in_maps = make_in_maps(
        x_type, x_value, seq_order, W_attn, type_emb, ln1_g, ln1_b,
        ln2_g, ln2_b, W1, b1, W2, b2,
    )
    res = run_bass_kernel_spmd(nc, in_maps, list(range(B)), trace=_trace)
    out = np.stack([res.results[i]["out"] for i in range(B)], axis=0)
    kernel.last_results = res
    return out


# revision 24
# speedup vs baseline: 1.3091x; 1.3091x over previous
"""Trainium2 Bass kernel for nn_ChemROAR (single transformer block, B=8).

Sharding: data-parallel over batch — core b computes batch element b.
No collectives. Matmuls run in float32r (rounded fp32, 1 cycle/row on PE
vs 4 for plain fp32).

Self-contained: only imports from /opt/trn_rl_repo (present on the target
machine image); no sibling files.
"""
import sys
import types

sys.path.insert(0, "/opt/trn_rl_repo")

import numpy as np

import concourse.bass as bass
import concourse.mybir as mybir
import concourse.tile as tile
import concourse.tile_utils as tile_utils
from concourse.vector_clock import ScopedClock

F32 = mybir.dt.float32
F32R = mybir.dt.float32r
BF16 = mybir.dt.bfloat16
I32 = mybir.dt.int32
AF = mybir.ActivationFunctionType
ALU = mybir.AluOpType

P = 128
B, T, D, H, DFF, NTYPE = 8, 1024, 512, 8, 1024, 341
HD = D // H          # 64
DPR = 32             # rotary dims per head
TT = T // P          # 8 token tiles
DK = D // P          # 4 d chunks
EPS = 1e-5
THETA = 10000.0
TWO_PI = 6.283185307179586
INV_2PI = 1.0 / TWO_PI
MAGIC = 12582912.0   # 1.5 * 2**23 — round-to-nearest magic for fp32
NH = HD + 1          # v columns + ones column (softmax denominator)
NCH = 2              # Tq chunks per head
CW = T // NCH        # 512

# SBUF cap: tile_utils default (192 KiB/partition) is stale; cayman has
# 208 KiB usable. Stay a bit under.
tile_utils.max_sbuf_usage = 207 * 1024

# ---------------------------------------------------------------------------
# Patch 1: the public walrus accepts only ONE attached sync-wait per
# instruction. Split excess waits onto standalone NoOps placed before the
# instruction (and split the kernel-tail drain into a chain of drains).
# ---------------------------------------------------------------------------
_MAXW = 1


def _install_tile_patch():
    if getattr(tile.TileContext, "_chemroar_patched", False):
        return
    orig_commit = tile.TileContext._commit_instruction

    def _commit_instruction(self, inst, lazy_reg_writes=True):
        si = getattr(inst, "sync_info", None)
        if si is not None and si.on_wait:
            waits = list(si.on_wait)
            if len(waits) > _MAXW:
                keep = waits[:_MAXW]
                excess = waits[_MAXW:]
                for i in range(0, len(excess), _MAXW):
                    nop = mybir.InstNoOp(
                        name=self.nc.get_next_instruction_name(),
                        ins=[],
                        outs=[],
                        sync_info=mybir.SyncInfo(
                            on_wait=excess[i : i + _MAXW], on_update=[]
                        ),
                        bass_nofuse=True,
                        engine=inst.engine,
                    )
                    self._add_instruction(nop)
                inst.sync_info = mybir.SyncInfo(
                    on_wait=keep, on_update=list(si.on_update)
                )
        return orig_commit(self, inst, lazy_reg_writes=lazy_reg_writes)

    def _drain_and_barrier(self, tick_clock, wait_clock):
        drain_inst = self.nc.sync.drain()
        wait_clock.add_sem_waits(
            drain_inst.ins, ScopedClock({None: tick_clock.global_clock})
        )
        mi = drain_inst.ins
        si = mi.sync_info
        if si is not None and si.on_wait and len(si.on_wait) > _MAXW:
            waits = list(si.on_wait)
            mi.sync_info = mybir.SyncInfo(
                on_wait=waits[:_MAXW], on_update=list(si.on_update)
            )
            for i in range(_MAXW, len(waits), _MAXW):
                d2 = self.nc.sync.drain()
                d2.ins.sync_info = mybir.SyncInfo(
                    on_wait=waits[i : i + _MAXW], on_update=[]
                )
        self.nc.all_engine_barrier()
        assert self.sems is not None
        popped = self.nc._tile_sem_poison_stack.pop()
        assert popped is self._sem_poison
        self.nc.clear_and_free_semaphores(list(self.sems.allocated().values()))
        self.nc.all_engine_barrier()

    tile.TileContext._commit_instruction = _commit_instruction
    tile.TileContext._drain_and_barrier = _drain_and_barrier
    tile.TileContext._chemroar_patched = True


_install_tile_patch()


# ---------------------------------------------------------------------------
# Patch 2: NTFF profile hook (the stripped antenv lacks axon_hooks).
# ---------------------------------------------------------------------------
def _install_hookfix():
    name = "antenv.axon_hooks"
    if name in sys.modules:
        return
    try:
        from trn_agent_boot.trn_boot import _ntff_profile_via_ctypes

        hook = _ntff_profile_via_ctypes("/opt/axon/libaxon_pjrt.so")
    except Exception:
        hook = None
    mod = types.ModuleType(name)
    mod._hook = hook
    mod.set_axon_ntff_profile_hook = lambda h: setattr(mod, "_hook", h)
    mod.get_axon_ntff_profile_hook = lambda: mod._hook
    sys.modules[name] = mod
    try:
        import antenv

        antenv.axon_hooks = mod
    except Exception:
        pass


_install_hookfix()


def _ap_with(a, offset_delta, ap_list):
    import dataclasses

    return dataclasses.replace(a, offset=a.offset + offset_delta, ap=ap_list)


def build_nc(trivial_ln1, trivial_ln2, trivial_b1, trivial_b2):
    nc = bass.Bass("TRN2", target_bir_lowering=False, debug=False)

    xv_d = nc.declare_dram_parameter("xv", [T, D], F32, isOutput=False)
    wa_d = nc.declare_dram_parameter("wa", [D, 3 * D], F32, isOutput=False)
    w1_d = nc.declare_dram_parameter("w1", [D, 2 * DFF], F32, isOutput=False)
    w2_d = nc.declare_dram_parameter("w2", [DFF, D], F32, isOutput=False)
    teq_d = nc.declare_dram_parameter("teq", [NTYPE, D], F32, isOutput=False)
    tek_d = nc.declare_dram_parameter("tek", [NTYPE, D], F32, isOutput=False)
    xtq_d = nc.declare_dram_parameter("xtq", [T], I32, isOutput=False)
    xtk_d = nc.declare_dram_parameter("xtk", [T], I32, isOutput=False)
    posq_d = nc.declare_dram_parameter("posq", [T], F32, isOutput=False)
    posk_d = nc.declare_dram_parameter("posk", [T], F32, isOutput=False)
    ident_d = nc.declare_dram_parameter("ident", [P, P], F32, isOutput=False)
    invf_d = nc.declare_dram_parameter("invf", [P, 16], F32, isOutput=False)
    g1_d = nc.declare_dram_parameter("g1", [D], F32, isOutput=False)
    b1ln_d = nc.declare_dram_parameter("b1ln", [D], F32, isOutput=False)
    g2_d = nc.declare_dram_parameter("g2", [D], F32, isOutput=False)
    b2ln_d = nc.declare_dram_parameter("b2ln", [D], F32, isOutput=False)
    bf1_d = nc.declare_dram_parameter("bf1", [2 * DFF], F32, isOutput=False)
    bf2_d = nc.declare_dram_parameter("bf2", [D], F32, isOutput=False)
    out_d = nc.declare_dram_parameter("out", [T, D], F32, isOutput=True)

    with tile.TileContext(nc) as tc:
        wpool = tc.alloc_tile_pool(name="wpool", bufs=1)
        work = tc.alloc_tile_pool(name="work", bufs=1)
        spool = tc.alloc_tile_pool(name="spool", bufs=2)
        psum = tc.alloc_tile_pool(name="psum", bufs=4, space="PSUM")
        psum_tr = tc.alloc_tile_pool(name="psum_tr", bufs=2, space="PSUM")
        psum_o = tc.alloc_tile_pool(name="psum_o", bufs=2, space="PSUM")

        # ---------------- small constants ----------------
        ident = wpool.tile([P, P], F32)
        nc.sync.dma_start(ident[:], ident_d.ap())
        identr = wpool.tile([P, P], F32R)
        nc.gpsimd.tensor_copy(identr[:], ident[:])

        invf = wpool.tile([P, 16], F32)
        nc.sync.dma_start(invf[:], invf_d.ap())

        posq_sb = wpool.tile([P, TT], F32)
        nc.sync.dma_start(posq_sb[:], posq_d.ap().rearrange("(a p) -> p a", p=P))
        posk_sb = wpool.tile([P, TT], F32)
        nc.sync.dma_start(posk_sb[:], posk_d.ap().rearrange("(a p) -> p a", p=P))

        if not trivial_b1:
            bf1_sb = wpool.tile([P, 2 * DFF // P], F32)
            nc.sync.dma_start(bf1_sb[:], bf1_d.ap().rearrange("(o p) -> p o", p=P))
        if not trivial_b2:
            bf2_sb = wpool.tile([P, DK], F32)
            nc.sync.dma_start(bf2_sb[:], bf2_d.ap().rearrange("(o p) -> p o", p=P))

        def load_weight_f32r(dram_ap, ko, n, tag, eng="gpsimd"):
            """DMA [ko*P, n] DRAM weight, cast to F32R via a scratch ring."""
            wr = work.tile([P, ko, n], F32R, tag=tag)
            src = dram_ap.rearrange("(ko ki) n -> ki ko n", ki=P)
            CHW = 512
            for k in range(ko):
                for c0 in range(0, n, CHW):
                    w = min(CHW, n - c0)
                    sc = spool.tile([P, CHW], F32, tag="wscratch", bufs=2)
                    nc.sync.dma_start(sc[:, :w], src[:, k, c0 : c0 + w])
                    if eng == "gpsimd":
                        nc.gpsimd.tensor_copy(wr[:, k, c0 : c0 + w], sc[:, :w])
                    else:
                        nc.vector.tensor_copy(wr[:, k, c0 : c0 + w], sc[:, :w])
            return wr

        # gamma/beta partition-broadcast tiles via K=1 matmul
        def bcast_row(src_dram, n, tag):
            row = wpool.tile([1, n], F32, tag=f"bcrow_{tag}")
            nc.sync.dma_start(row[:], src_dram.ap().rearrange("(o n) -> o n", o=1))
            rowr = wpool.tile([1, n], F32R, tag=f"bcrowr_{tag}")
            nc.vector.tensor_copy(rowr[:], row[:])
            onesc = wpool.tile([1, P], F32R, tag="bc_ones")
            nc.vector.memset(onesc[:], 1.0)
            out_t = wpool.tile([P, n], F32, tag=f"bcout_{tag}")
            for c0 in range(0, n, 512):
                w = min(512, n - c0)
                pt = psum_o.tile([P, CW], F32, tag="o_ps")
                nc.tensor.matmul(
                    pt[:, :w], lhsT=onesc[:], rhs=rowr[:, c0 : c0 + w],
                    start=True, stop=True,
                )
                nc.scalar.copy(out_t[:, c0 : c0 + w], pt[:, :w])
            return out_t

        g1_bc = b1_bc = g2_bc = b2_bc = None
        if not trivial_ln1:
            g1_bc = bcast_row(g1_d, D, "g1")
            b1_bc = bcast_row(b1ln_d, D, "b1")
        if not trivial_ln2:
            g2_bc = bcast_row(g2_d, D, "g2")
            b2_bc = bcast_row(b2ln_d, D, "b2")

        # attention weights now; FFN weights later (lifetime-shared tags)
        war = load_weight_f32r(wa_d.ap(), DK, 3 * D, tag="w_big")

        # ---------------- load x ----------------
        xs = work.tile([P, TT, D], F32, tag="xs_h2T")
        for ti in range(TT):
            nc.sync.dma_start(xs[:, ti, :], xv_d.ap()[ti * P : (ti + 1) * P, :])

        # ---------------- helpers ----------------
        def layernorm_tile(x_ap, out_ap, g_bc, b_bc, trivial):
            m = spool.tile([P, 1], F32, tag="ln_m")
            nc.vector.reduce_sum(m[:], x_ap, axis=mybir.AxisListType.X)
            nc.vector.tensor_scalar_mul(m[:], m[:], 1.0 / D)
            sq = spool.tile([P, 1], F32, tag="ln_sq")
            # out_ap doubles as junk output for the squared pass
            nc.scalar.activation(out_ap, x_ap, AF.Square, accum_out=sq[:])
            mm2 = spool.tile([P, 1], F32, tag="ln_mm2")
            nc.vector.tensor_tensor(mm2[:], m[:], m[:], ALU.mult)
            s = spool.tile([P, 1], F32, tag="ln_s")
            nc.vector.tensor_scalar(s[:], sq[:], 1.0 / D, EPS, ALU.mult, ALU.add)
            nc.vector.tensor_tensor(s[:], s[:], mm2[:], ALU.subtract)
            nc.vector.reciprocal(s[:], s[:])
            nc.scalar.sqrt(s[:], s[:])
            if trivial:
                nc.vector.tensor_scalar(out_ap, x_ap, m[:], s[:],
                                        ALU.subtract, ALU.mult)
            else:
                tmp = spool.tile([P, D], F32, tag="ring_eq_sig")
                nc.vector.tensor_scalar(tmp[:], x_ap, m[:], s[:],
                                        ALU.subtract, ALU.mult)
                nc.vector.tensor_tensor(tmp[:], tmp[:], g_bc[:], ALU.mult)
                nc.vector.tensor_tensor(out_ap, tmp[:], b_bc[:], ALU.add)

        _tr_flip = [0]

        def transpose_128(src_ap, dst_ap):
            pt = psum_tr.tile([P, P], F32R, tag="tr_ps")
            nc.tensor.transpose(pt[:], src_ap, identr[:])
            _tr_flip[0] ^= 1
            if _tr_flip[0]:
                nc.vector.tensor_copy(dst_ap, pt[:])
            else:
                nc.scalar.copy(dst_ap, pt[:])

        # ---------------- LN1 + transpose h (per-tile ring) --------------
        hT = work.tile([P, DK, T], F32R, tag="hT_qT")
        for ti in range(TT):
            h_t = spool.tile([P, D], F32R, tag="h_ring_f")
            layernorm_tile(xs[:, ti, :], h_t[:], g1_bc, b1_bc, trivial_ln1)
            for j in range(DK):
                transpose_128(
                    h_t[:, j * P : (j + 1) * P],
                    hT[:, j, ti * P : (ti + 1) * P],
                )

        # ---------------- qkv + emb + rope + transpose --------------------
        # NOTE: qT shares the hT tag slot, so allocate it only after hT's
        # last use. We therefore first compute q_sb/k_sb/vext fully.
        q_sb = work.tile([P, TT, D], F32R, tag="q_sb")
        k_sb = work.tile([P, TT, D], F32R, tag="k_sb")
        vext = work.tile([P, TT, H, NH], BF16, tag="vext_w2")
        onesf = wpool.tile([P, H], F32, tag="onesf")
        nc.gpsimd.memset(onesf[:], 1.0)
        for ti in range(TT):
            nc.gpsimd.tensor_copy(
                vext[:, ti, :, HD : HD + 1],
                onesf[:].rearrange("p (h o) -> p h o", o=1),
            )

        for ti in range(TT):
            # emb gathers for this tile
            offq = spool.tile([P, 1], I32, tag="offq")
            nc.sync.dma_start(
                offq[:], xtq_d.ap()[ti * P : (ti + 1) * P].rearrange("(p o) -> p o", o=1)
            )
            eq = spool.tile([P, D], F32, tag="ring_eq_sig")
            nc.gpsimd.indirect_dma_start(
                out=eq[:],
                out_offset=None,
                in_=teq_d.ap(),
                in_offset=bass.IndirectOffsetOnAxis(ap=offq[:], axis=0),
            )
            offk = spool.tile([P, 1], I32, tag="offk")
            nc.sync.dma_start(
                offk[:], xtk_d.ap()[ti * P : (ti + 1) * P].rearrange("(p o) -> p o", o=1)
            )
            ek = spool.tile([P, D], F32, tag="ek_ring")
            nc.gpsimd.indirect_dma_start(
                out=ek[:],
                out_offset=None,
                in_=tek_d.ap(),
                in_offset=bass.IndirectOffsetOnAxis(ap=offk[:], axis=0),
            )
            pts = {}
            for which in ("q", "k", "v"):
                pts[which] = psum.tile([P, CW], F32, tag="mm_ps", name=f"qkv_{which}")
            for kk in range(DK):
                for wi, (which, base) in enumerate(
                    (("q", 0), ("k", D), ("v", 2 * D))
                ):
                    nc.tensor.matmul(
                        pts[which][:, :D],
                        lhsT=hT[:, kk, ti * P : (ti + 1) * P],
                        rhs=war[:, kk, base : base + D],
                        start=(kk == 0),
                        stop=(kk == DK - 1),
                    )
            nc.vector.tensor_tensor(q_sb[:, ti, :], pts["q"][:, :D], eq[:], ALU.add)
            nc.vector.tensor_tensor(k_sb[:, ti, :], pts["k"][:, :D], ek[:], ALU.add)
            nc.vector.tensor_copy(
                vext[:, ti, :, 0:HD],
                pts["v"][:, :D].rearrange("p (h x) -> p h x", h=H),
            )

        # ---------------- RoPE (token-major, in place) --------------------
        def rope_tile(dst, ti, pos_sb):
            fr = spool.tile([P, 16], F32, tag="rp_fr")
            nc.vector.tensor_scalar_mul(fr[:], invf[:], pos_sb[:, ti : ti + 1])

            def lut_arg(tag, quarter):
                y = spool.tile([P, 16], F32, tag=f"rp_y{tag}")
                nc.vector.tensor_scalar(
                    y[:], fr[:], INV_2PI, 0.25 if quarter else 0.0,
                    ALU.mult, ALU.add,
                )
                kk = spool.tile([P, 16], F32, tag=f"rp_k{tag}")
                nc.vector.tensor_scalar(
                    kk[:], y[:], MAGIC, MAGIC, ALU.add, ALU.subtract
                )
                ang = spool.tile([P, 16], F32, tag=f"rp_a{tag}")
                nc.vector.scalar_tensor_tensor(
                    ang[:], kk[:], -TWO_PI, fr[:], ALU.mult, ALU.add
                )
                if quarter:
                    nc.vector.tensor_scalar_add(ang[:], ang[:], np.pi / 2)
                sc = spool.tile([P, 16], F32, tag=f"rp_s{tag}")
                nc.scalar.activation(sc[:], ang[:], AF.Sin)
                return sc

            sin16 = lut_arg("s", False)
            cos16 = lut_arg("c", True)
            cos32 = spool.tile([P, 32], F32, tag="rp_cos32")
            c32v = cos32[:].rearrange("p (u v) -> p u v", v=2)
            nc.vector.tensor_copy(c32v[:, :, 0], cos16[:])
            nc.vector.tensor_copy(c32v[:, :, 1], cos16[:])
            sin32 = spool.tile([P, 32], F32, tag="rp_sin32")
            s32v = sin32[:].rearrange("p (u v) -> p u v", v=2)
            nc.scalar.mul(s32v[:, :, 0], sin16[:], -1.0)
            nc.vector.tensor_copy(s32v[:, :, 1], sin16[:])

            rot = (
                dst[:, ti, :]
                .rearrange("p (h x) -> p h x", h=H)[:, :, 0:DPR]
                .rearrange("p h (u v) -> p h u v", v=2)
            )
            shuf = _ap_with(rot, 1, [rot.ap[0], rot.ap[1], rot.ap[2], [-1, 2]])
            sin_b = (
                sin32[:].rearrange("p (u v) -> p u v", v=2)
                .unsqueeze(1)
                .broadcast_to((P, H, 16, 2))
            )
            cos_b = (
                cos32[:].rearrange("p (u v) -> p u v", v=2)
                .unsqueeze(1)
                .broadcast_to((P, H, 16, 2))
            )
            tmp = spool.tile([P, H, 16, 2], BF16, tag="rp_tmp")
            nc.vector.tensor_tensor(tmp[:], shuf, sin_b, ALU.mult)
            nc.vector.tensor_tensor(rot, rot, cos_b, ALU.mult)
            nc.vector.tensor_tensor(rot, rot, tmp[:], ALU.add)

        for ti in range(TT):
            rope_tile(q_sb, ti, posq_sb)
            rope_tile(k_sb, ti, posk_sb)

        # ---------------- transpose q, k (qT reuses hT slot) --------------
        qT = work.tile([P, DK, T], BF16, tag="hT_qT")
        kT = work.tile([P, DK, T], BF16, tag="kT_gT")
        for ti in range(TT):
            for j in range(DK):
                transpose_128(
                    q_sb[:, ti, j * P : (j + 1) * P],
                    qT[:, j, ti * P : (ti + 1) * P],
                )
                transpose_128(
                    k_sb[:, ti, j * P : (j + 1) * P],
                    kT[:, j, ti * P : (ti + 1) * P],
                )

        # ---------------- attention ----------------
        x_new = work.tile([P, TT, D], F32, tag="x_new")
        for j in range(H // 2):
            expTs = []
            for sub in range(2):
                expTs.append(
                    work.tile([P, TT, CW], BF16, tag="expT_bufs", bufs=2,
                              name=f"expT_{j}_{sub}")
                )
            oTs = []
            for sub in range(2):
                oTs.append(
                    work.tile([NH, T], F32, tag="oT", bufs=2,
                              name=f"oT_{j}_{sub}")
                )
            for c in range(NCH):
                lim = 4 * c + 4
                for ti in range(lim):
                    pss = []
                    for sub in range(2):
                        r0 = 64 * sub
                        ps = psum.tile([P, CW], F32, tag="mm_ps",
                                       name=f"sc_{j}_{sub}")
                        # heads 2j (rows 0-63) and 2j+1 (rows 64-127) run
                        # concurrently on disjoint PE row groups
                        nc.tensor.matmul(
                            ps[:],
                            lhsT=kT[r0 : r0 + HD, j, ti * P : (ti + 1) * P],
                            rhs=qT[r0 : r0 + HD, j, c * CW : (c + 1) * CW],
                            start=True,
                            stop=True,
                        )
                        pss.append(ps)
                    off = P * (ti - 4 * c)
                    for sub in range(2):
                        expT = expTs[sub]
                        ps = pss[sub]
                        if off <= -P:
                            nc.scalar.activation(
                                expT[:, ti, :], ps[:], AF.Exp, scale=0.125
                            )
                        else:
                            nc.scalar.activation(
                                expT[:, ti, off:CW], ps[:, off:CW], AF.Exp,
                                scale=0.125,
                            )
                            if off > 0:
                                nc.gpsimd.memset(expT[:, ti, 0:off], 0.0)
                            nc.gpsimd.affine_select(
                                out=expT[:, ti, off : off + P],
                                in_=expT[:, ti, off : off + P],
                                pattern=[[1, P]],
                                compare_op=ALU.is_ge,
                                fill=0.0,
                                base=0,
                                channel_multiplier=-1,
                            )
                pos = []
                for sub in range(2):
                    pos.append(psum_o.tile([P, CW], F32, tag="o_ps",
                                           name=f"po_{j}_{sub}"))
                for ti in range(lim):
                    for sub in range(2):
                        nc.tensor.matmul(
                            pos[sub][0:NH, :],
                            lhsT=vext[:, ti, 2 * j + sub, :],
                            rhs=expTs[sub][:, ti, :],
                            start=(ti == 0),
                            stop=(ti == lim - 1),
                        )
                for sub in range(2):
                    nc.vector.tensor_copy(
                        oTs[sub][:, c * CW : (c + 1) * CW], pos[sub][0:NH, :]
                    )
            for ti in range(TT):
                for sub in range(2):
                    hh = 2 * j + sub
                    pt = psum_tr.tile([P, P], F32, tag="tr_ps",
                                      name=f"tro_{j}_{sub}")
                    nc.tensor.matmul(
                        pt[:, 0:NH],
                        lhsT=oTs[sub][:, ti * P : (ti + 1) * P],
                        rhs=ident[0:NH, 0:NH],
                        is_transpose=True,
                        start=True,
                        stop=True,
                    )
                    rec = spool.tile([P, 1], F32, tag="rec")
                    nc.vector.reciprocal(rec[:], pt[:, HD : HD + 1])
                    nc.vector.scalar_tensor_tensor(
                        x_new[:, ti, hh * HD : (hh + 1) * HD],
                        pt[:, 0:HD],
                        rec[:],
                        xs[:, ti, hh * HD : (hh + 1) * HD],
                        ALU.mult,
                        ALU.add,
                    )

        # ---------------- FFN weights (reuse attention weight slots) ------
        w1r = load_weight_f32r(w1_d.ap(), DK, 2 * DFF, tag="w_big", eng="vector")
        w2r = load_weight_f32r(w2_d.ap(), DFF // P, D, tag="vext_w2", eng="vector")

        # ---------------- LN2 + transpose h2 (h2T reuses xs slot) ---------
        h2T = work.tile([P, DK, T], F32R, tag="xs_h2T")
        for ti in range(TT):
            h2_t = spool.tile([P, D], F32R, tag="h_ring_f")
            layernorm_tile(x_new[:, ti, :], h2_t[:], g2_bc, b2_bc, trivial_ln2)
            for j in range(DK):
                transpose_128(
                    h2_t[:, j * P : (j + 1) * P],
                    h2T[:, j, ti * P : (ti + 1) * P],
                )

        # ---------------- FFN ----------------
        gT = work.tile([P, DFF // P, T], F32R, tag="kT_gT")
        for m in range(DFF // P):
            sg = spool.tile([P, CW], F32, tag="ring_eq_sig")
            for c in range(NCH):
                pa = psum.tile([P, CW], F32, tag="mm_ps")
                pg = psum.tile([P, CW], F32, tag="mm_ps")
                for kk in range(DK):
                    nc.tensor.matmul(
                        pa[:],
                        lhsT=w1r[:, kk, m * P : (m + 1) * P],
                        rhs=h2T[:, kk, c * CW : (c + 1) * CW],
                        start=(kk == 0),
                        stop=(kk == DK - 1),
                    )
                for kk in range(DK):
                    nc.tensor.matmul(
                        pg[:],
                        lhsT=w1r[:, kk, DFF + m * P : DFF + (m + 1) * P],
                        rhs=h2T[:, kk, c * CW : (c + 1) * CW],
                        start=(kk == 0),
                        stop=(kk == DK - 1),
                    )
                cs = slice(c * CW, (c + 1) * CW)
                if trivial_b1:
                    nc.scalar.activation(sg[:], pg[:], AF.Sigmoid)
                    nc.vector.tensor_tensor(sg[:], pg[:], sg[:], ALU.mult)
                    nc.vector.tensor_tensor(gT[:, m, cs], pa[:], sg[:], ALU.mult)
                else:
                    bgap = bf1_sb[:, (DFF // P) + m : (DFF // P) + m + 1]
                    nc.scalar.activation(sg[:], pg[:], AF.Sigmoid, bias=bgap)
                    nc.vector.scalar_tensor_tensor(
                        sg[:], pg[:], bgap, sg[:], ALU.add, ALU.mult
                    )
                    nc.vector.scalar_tensor_tensor(
                        gT[:, m, cs], pa[:], bf1_sb[:, m : m + 1], sg[:],
                        ALU.add, ALU.mult,
                    )

        yT = work.tile([P, DK, T], F32R, tag="q_sb")
        for m in range(DK):
            for c in range(NCH):
                py = psum.tile([P, CW], F32, tag="mm_ps")
                for kk in range(DFF // P):
                    nc.tensor.matmul(
                        py[:],
                        lhsT=w2r[:, kk, m * P : (m + 1) * P],
                        rhs=gT[:, kk, c * CW : (c + 1) * CW],
                        start=(kk == 0),
                        stop=(kk == DFF // P - 1),
                    )
                cs = slice(c * CW, (c + 1) * CW)
                if trivial_b2:
                    nc.scalar.copy(yT[:, m, cs], py[:])
                else:
                    nc.vector.tensor_scalar_add(yT[:, m, cs], py[:], bf2_sb[:, m : m + 1])

        # ---------------- final transpose + residual + store -------------
        for ti in range(TT):
            fin = spool.tile([P, D], F32, tag="h_ring_f")
            for j in range(DK):
                pt = psum_tr.tile([P, P], F32R, tag="tr_ps")
                nc.tensor.transpose(pt[:], yT[:, j, ti * P : (ti + 1) * P], identr[:])
                nc.vector.tensor_tensor(
                    fin[:, j * P : (j + 1) * P],
                    pt[:],
                    x_new[:, ti, j * P : (j + 1) * P],
                    ALU.add,
                )
            nc.sync.dma_start(out_d.ap()[ti * P : (ti + 1) * P, :], fin[:])

        for p in (psum_o, psum_tr, psum, spool, work, wpool):
            p.release()

    return nc


_CACHE = {}


def _get_nc(key):
    if key not in _CACHE:
        _CACHE[key] = build_nc(*key)
    return _CACHE[key]


def make_in_maps(x_type, x_value, seq_order, W_attn, type_emb, ln1_g, ln1_b,
                 ln2_g, ln2_b, W1, b1, W2, b2):
    ident = np.eye(P, dtype=np.float32)
    inv_freq = 1.0 / (THETA ** (np.arange(0, DPR, 2, dtype=np.float32) / DPR))
    invf = np.tile(inv_freq[None, :], (P, 1)).astype(np.float32)
    in_maps = []
    for b in range(B):
        in_maps.append({
            "xv": np.ascontiguousarray(x_value[b], dtype=np.float32),
            "wa": np.asarray(W_attn, dtype=np.float32),
            "w1": np.asarray(W1, dtype=np.float32),
            "w2": np.asarray(W2, dtype=np.float32),
            "teq": np.ascontiguousarray(type_emb[:, :D], dtype=np.float32),
            "tek": np.ascontiguousarray(type_emb[:, D:], dtype=np.float32),
            "xtq": np.ascontiguousarray(x_type[b, :T], dtype=np.int32),
            "xtk": np.ascontiguousarray(x_type[b, 1 : T + 1], dtype=np.int32),
            "posq": np.ascontiguousarray(seq_order[b, :T], dtype=np.float32),
            "posk": np.ascontiguousarray(seq_order[b, 1 : T + 1], dtype=np.float32),
            "ident": ident,
            "invf": invf,
            "g1": np.asarray(ln1_g, dtype=np.float32),
            "b1ln": np.asarray(ln1_b, dtype=np.float32),
            "g2": np.asarray(ln2_g, dtype=np.float32),
            "b2ln": np.asarray(ln2_b, dtype=np.float32),
            "bf1": np.asarray(b1, dtype=np.float32),
            "bf2": np.asarray(b2, dtype=np.float32),
        })
    return in_maps


def triviality_key(ln1_g, ln1_b, ln2_g, ln2_b, b1, b2):
    return (
        bool(np.all(np.asarray(ln1_g) == 1.0) and np.all(np.asarray(ln1_b) == 0.0)),
        bool(np.all(np.asarray(ln2_g) == 1.0) and np.all(np.asarray(ln2_b) == 0.0)),
        bool(np.all(np.asarray(b1) == 0.0)),
        bool(np.all(np.asarray(b2) == 0.0)),
    )


def kernel(x_type, x_value, seq_order, W_attn, type_emb, ln1_g, ln1_b,
           ln2_g, ln2_b, W1, b1, W2, b2, _trace=False):
    from concourse.bass_utils import run_bass_kernel_spmd

    key = triviality_key(ln1_g, ln1_b, ln2_g, ln2_b, b1, b2)
    nc = _get_nc(key)
    in_maps = make_in_maps(
        x_type, x_value, seq_order, W_attn, type_emb, ln1_g, ln1_b,
        ln2_g, ln2_b, W1, b1, W2, b2,
    )
    res = run_bass_kernel_spmd(nc, in_maps, list(range(B)), trace=_trace)
    out = np.stack([res.results[i]["out"] for i in range(B)], axis=0)
    kernel.last_results = res
    return out


# revision 28
# speedup vs baseline: 1.3592x; 1.0383x over previous
"""Trainium2 Bass kernel for nn_ChemROAR (single transformer block, B=8).

Sharding: data-parallel over batch — core b computes batch element b.
No collectives. Matmuls run in float32r (rounded fp32, 1 cycle/row on PE
vs 4 for plain fp32).

Self-contained: only imports from /opt/trn_rl_repo (present on the target
machine image); no sibling files.
"""
import sys
import types

sys.path.insert(0, "/opt/trn_rl_repo")

import numpy as np

import concourse.bass as bass
import concourse.mybir as mybir
import concourse.tile as tile
import concourse.tile_utils as tile_utils
from concourse.vector_clock import ScopedClock

F32 = mybir.dt.float32
F32R = mybir.dt.float32r
BF16 = mybir.dt.bfloat16
I32 = mybir.dt.int32
AF = mybir.ActivationFunctionType
ALU = mybir.AluOpType

P = 128
B, T, D, H, DFF, NTYPE = 8, 1024, 512, 8, 1024, 341
HD = D // H          # 64
DPR = 32             # rotary dims per head
TT = T // P          # 8 token tiles
DK = D // P          # 4 d chunks
EPS = 1e-5
THETA = 10000.0
TWO_PI = 6.283185307179586
INV_2PI = 1.0 / TWO_PI
MAGIC = 12582912.0   # 1.5 * 2**23 — round-to-nearest magic for fp32
NH = HD + 1          # v columns + ones column (softmax denominator)
NCH = 2              # Tq chunks per head
CW = T // NCH        # 512

# SBUF cap: tile_utils default (192 KiB/partition) is stale; cayman has
# 208 KiB usable. Stay a bit under.
tile_utils.max_sbuf_usage = 207 * 1024

# ---------------------------------------------------------------------------
# Patch 1: the public walrus accepts only ONE attached sync-wait per
# instruction. Split excess waits onto standalone NoOps placed before the
# instruction (and split the kernel-tail drain into a chain of drains).
# ---------------------------------------------------------------------------
_MAXW = 1


def _install_tile_patch():
    if getattr(tile.TileContext, "_chemroar_patched", False):
        return
    orig_commit = tile.TileContext._commit_instruction

    def _commit_instruction(self, inst, lazy_reg_writes=True):
        si = getattr(inst, "sync_info", None)
        if si is not None and si.on_wait:
            waits = list(si.on_wait)
            if len(waits) > _MAXW:
                keep = waits[:_MAXW]
                excess = waits[_MAXW:]
                for i in range(0, len(excess), _MAXW):
                    nop = mybir.InstNoOp(
                        name=self.nc.get_next_instruction_name(),
                        ins=[],
                        outs=[],
                        sync_info=mybir.SyncInfo(
                            on_wait=excess[i : i + _MAXW], on_update=[]
                        ),
                        bass_nofuse=True,
                        engine=inst.engine,
                    )
                    self._add_instruction(nop)
                inst.sync_info = mybir.SyncInfo(
                    on_wait=keep, on_update=list(si.on_update)
                )
        return orig_commit(self, inst, lazy_reg_writes=lazy_reg_writes)

    def _drain_and_barrier(self, tick_clock, wait_clock):
        drain_inst = self.nc.sync.drain()
        wait_clock.add_sem_waits(
            drain_inst.ins, ScopedClock({None: tick_clock.global_clock})
        )
        mi = drain_inst.ins
        si = mi.sync_info
        if si is not None and si.on_wait and len(si.on_wait) > _MAXW:
            waits = list(si.on_wait)
            mi.sync_info = mybir.SyncInfo(
                on_wait=waits[:_MAXW], on_update=list(si.on_update)
            )
            for i in range(_MAXW, len(waits), _MAXW):
                d2 = self.nc.sync.drain()
                d2.ins.sync_info = mybir.SyncInfo(
                    on_wait=waits[i : i + _MAXW], on_update=[]
                )
        self.nc.all_engine_barrier()
        assert self.sems is not None
        popped = self.nc._tile_sem_poison_stack.pop()
        assert popped is self._sem_poison
        self.nc.clear_and_free_semaphores(list(self.sems.allocated().values()))
        self.nc.all_engine_barrier()

    tile.TileContext._commit_instruction = _commit_instruction
    tile.TileContext._drain_and_barrier = _drain_and_barrier
    tile.TileContext._chemroar_patched = True


_install_tile_patch()


# ---------------------------------------------------------------------------
# Patch 2: NTFF profile hook (the stripped antenv lacks axon_hooks).
# ---------------------------------------------------------------------------
def _install_hookfix():
    name = "antenv.axon_hooks"
    if name in sys.modules:
        return
    try:
        from trn_agent_boot.trn_boot import _ntff_profile_via_ctypes

        hook = _ntff_profile_via_ctypes("/opt/axon/libaxon_pjrt.so")
    except Exception:
        hook = None
    mod = types.ModuleType(name)
    mod._hook = hook
    mod.set_axon_ntff_profile_hook = lambda h: setattr(mod, "_hook", h)
    mod.get_axon_ntff_profile_hook = lambda: mod._hook
    sys.modules[name] = mod
    try:
        import antenv

        antenv.axon_hooks = mod
    except Exception:
        pass


_install_hookfix()


def _ap_with(a, offset_delta, ap_list):
    import dataclasses

    return dataclasses.replace(a, offset=a.offset + offset_delta, ap=ap_list)


def build_nc(trivial_ln1, trivial_ln2, trivial_b1, trivial_b2):
    nc = bass.Bass("TRN2", target_bir_lowering=False, debug=False)

    xv_d = nc.declare_dram_parameter("xv", [T, D], F32, isOutput=False)
    wa_d = nc.declare_dram_parameter("wa", [D, 3 * D], F32, isOutput=False)
    w1_d = nc.declare_dram_parameter("w1", [D, 2 * DFF], F32, isOutput=False)
    w2_d = nc.declare_dram_parameter("w2", [DFF, D], F32, isOutput=False)
    teq_d = nc.declare_dram_parameter("teq", [NTYPE, D], F32, isOutput=False)
    tek_d = nc.declare_dram_parameter("tek", [NTYPE, D], F32, isOutput=False)
    xtq_d = nc.declare_dram_parameter("xtq", [T], I32, isOutput=False)
    xtk_d = nc.declare_dram_parameter("xtk", [T], I32, isOutput=False)
    posq_d = nc.declare_dram_parameter("posq", [T], F32, isOutput=False)
    posk_d = nc.declare_dram_parameter("posk", [T], F32, isOutput=False)
    ident_d = nc.declare_dram_parameter("ident", [P, P], F32, isOutput=False)
    invf_d = nc.declare_dram_parameter("invf", [P, 16], F32, isOutput=False)
    g1_d = nc.declare_dram_parameter("g1", [D], F32, isOutput=False)
    b1ln_d = nc.declare_dram_parameter("b1ln", [D], F32, isOutput=False)
    g2_d = nc.declare_dram_parameter("g2", [D], F32, isOutput=False)
    b2ln_d = nc.declare_dram_parameter("b2ln", [D], F32, isOutput=False)
    bf1_d = nc.declare_dram_parameter("bf1", [2 * DFF], F32, isOutput=False)
    bf2_d = nc.declare_dram_parameter("bf2", [D], F32, isOutput=False)
    out_d = nc.declare_dram_parameter("out", [T, D], F32, isOutput=True)

    with tile.TileContext(nc) as tc:
        wpool = tc.alloc_tile_pool(name="wpool", bufs=1)
        work = tc.alloc_tile_pool(name="work", bufs=1)
        spool = tc.alloc_tile_pool(name="spool", bufs=2)
        psum = tc.alloc_tile_pool(name="psum", bufs=4, space="PSUM")
        psum_tr = tc.alloc_tile_pool(name="psum_tr", bufs=2, space="PSUM")
        psum_o = tc.alloc_tile_pool(name="psum_o", bufs=2, space="PSUM")

        # ---------------- small constants ----------------
        ident = wpool.tile([P, P], F32)
        nc.sync.dma_start(ident[:], ident_d.ap())
        identr = wpool.tile([P, P], F32R)
        nc.gpsimd.tensor_copy(identr[:], ident[:])

        invf = wpool.tile([P, 16], F32)
        nc.sync.dma_start(invf[:], invf_d.ap())

        posq_sb = wpool.tile([P, TT], F32)
        nc.sync.dma_start(posq_sb[:], posq_d.ap().rearrange("(a p) -> p a", p=P))
        posk_sb = wpool.tile([P, TT], F32)
        nc.sync.dma_start(posk_sb[:], posk_d.ap().rearrange("(a p) -> p a", p=P))

        if not trivial_b1:
            bf1_sb = wpool.tile([P, 2 * DFF // P], F32)
            nc.sync.dma_start(bf1_sb[:], bf1_d.ap().rearrange("(o p) -> p o", p=P))
        if not trivial_b2:
            bf2_sb = wpool.tile([P, DK], F32)
            nc.sync.dma_start(bf2_sb[:], bf2_d.ap().rearrange("(o p) -> p o", p=P))

        def load_weight_f32r(dram_ap, ko, n, tag, eng="gpsimd"):
            """DMA [ko*P, n] DRAM weight, cast to F32R via a scratch ring."""
            wr = work.tile([P, ko, n], F32R, tag=tag)
            src = dram_ap.rearrange("(ko ki) n -> ki ko n", ki=P)
            CHW = 512
            for k in range(ko):
                for c0 in range(0, n, CHW):
                    w = min(CHW, n - c0)
                    sc = spool.tile([P, CHW], F32, tag="wscratch", bufs=2)
                    nc.sync.dma_start(sc[:, :w], src[:, k, c0 : c0 + w])
                    if eng == "gpsimd":
                        nc.gpsimd.tensor_copy(wr[:, k, c0 : c0 + w], sc[:, :w])
                    else:
                        nc.vector.tensor_copy(wr[:, k, c0 : c0 + w], sc[:, :w])
            return wr

        # gamma/beta partition-broadcast tiles via K=1 matmul
        def bcast_row(src_dram, n, tag):
            row = wpool.tile([1, n], F32, tag=f"bcrow_{tag}")
            nc.sync.dma_start(row[:], src_dram.ap().rearrange("(o n) -> o n", o=1))
            rowr = wpool.tile([1, n], F32R, tag=f"bcrowr_{tag}")
            nc.vector.tensor_copy(rowr[:], row[:])
            onesc = wpool.tile([1, P], F32R, tag="bc_ones")
            nc.vector.memset(onesc[:], 1.0)
            out_t = wpool.tile([P, n], F32, tag=f"bcout_{tag}")
            for c0 in range(0, n, 512):
                w = min(512, n - c0)
                pt = psum_o.tile([P, CW], F32, tag="o_ps")
                nc.tensor.matmul(
                    pt[:, :w], lhsT=onesc[:], rhs=rowr[:, c0 : c0 + w],
                    start=True, stop=True,
                )
                nc.scalar.copy(out_t[:, c0 : c0 + w], pt[:, :w])
            return out_t

        g1_bc = b1_bc = g2_bc = b2_bc = None
        if not trivial_ln1:
            g1_bc = bcast_row(g1_d, D, "g1")
            b1_bc = bcast_row(b1ln_d, D, "b1")
        if not trivial_ln2:
            g2_bc = bcast_row(g2_d, D, "g2")
            b2_bc = bcast_row(b2ln_d, D, "b2")

        # attention weights now; FFN weights later (lifetime-shared tags)
        war = load_weight_f32r(wa_d.ap(), DK, 3 * D, tag="w_big")

        # ---------------- load x ----------------
        xs = work.tile([P, TT, D], F32, tag="xs_h2T")
        for ti in range(TT):
            nc.sync.dma_start(xs[:, ti, :], xv_d.ap()[ti * P : (ti + 1) * P, :])

        # ---------------- helpers ----------------
        def layernorm_tile(x_ap, out_ap, g_bc, b_bc, trivial):
            m = spool.tile([P, 1], F32, tag="ln_m")
            nc.vector.reduce_sum(m[:], x_ap, axis=mybir.AxisListType.X)
            nc.vector.tensor_scalar_mul(m[:], m[:], 1.0 / D)
            sq = spool.tile([P, 1], F32, tag="ln_sq")
            # out_ap doubles as junk output for the squared pass
            nc.scalar.activation(out_ap, x_ap, AF.Square, accum_out=sq[:])
            mm2 = spool.tile([P, 1], F32, tag="ln_mm2")
            nc.vector.tensor_tensor(mm2[:], m[:], m[:], ALU.mult)
            s = spool.tile([P, 1], F32, tag="ln_s")
            nc.vector.tensor_scalar(s[:], sq[:], 1.0 / D, EPS, ALU.mult, ALU.add)
            nc.vector.tensor_tensor(s[:], s[:], mm2[:], ALU.subtract)
            nc.vector.reciprocal(s[:], s[:])
            nc.scalar.sqrt(s[:], s[:])
            if trivial:
                nc.vector.tensor_scalar(out_ap, x_ap, m[:], s[:],
                                        ALU.subtract, ALU.mult)
            else:
                tmp = spool.tile([P, D], F32, tag="ring_eq_sig")
                nc.vector.tensor_scalar(tmp[:], x_ap, m[:], s[:],
                                        ALU.subtract, ALU.mult)
                nc.vector.tensor_tensor(tmp[:], tmp[:], g_bc[:], ALU.mult)
                nc.vector.tensor_tensor(out_ap, tmp[:], b_bc[:], ALU.add)

        _tr_flip = [0]

        def transpose_128(src_ap, dst_ap):
            pt = psum_tr.tile([P, P], F32R, tag="tr_ps")
            nc.tensor.transpose(pt[:], src_ap, identr[:])
            _tr_flip[0] ^= 1
            if _tr_flip[0]:
                nc.vector.tensor_copy(dst_ap, pt[:])
            else:
                nc.scalar.copy(dst_ap, pt[:])

        # ---------------- LN1 + transpose h (per-tile ring) --------------
        hT = work.tile([P, DK, T], F32R, tag="hT_qT")
        for ti in range(TT):
            h_t = spool.tile([P, D], F32R, tag="h_ring_f")
            layernorm_tile(xs[:, ti, :], h_t[:], g1_bc, b1_bc, trivial_ln1)
            for j in range(DK):
                transpose_128(
                    h_t[:, j * P : (j + 1) * P],
                    hT[:, j, ti * P : (ti + 1) * P],
                )

        # ---------------- qkv + emb + rope + transpose --------------------
        # NOTE: qT shares the hT tag slot, so allocate it only after hT's
        # last use. We therefore first compute q_sb/k_sb/vext fully.
        q_sb = work.tile([P, TT, D], F32R, tag="q_sb")
        k_sb = work.tile([P, TT, D], F32R, tag="k_sb")
        vext = work.tile([P, TT, H, NH], BF16, tag="vext_w2")
        onesf = wpool.tile([P, H], F32, tag="onesf")
        nc.gpsimd.memset(onesf[:], 1.0)
        for ti in range(TT):
            nc.gpsimd.tensor_copy(
                vext[:, ti, :, HD : HD + 1],
                onesf[:].rearrange("p (h o) -> p h o", o=1),
            )

        for ti in range(TT):
            # emb gathers for this tile
            offq = spool.tile([P, 1], I32, tag="offq")
            nc.sync.dma_start(
                offq[:], xtq_d.ap()[ti * P : (ti + 1) * P].rearrange("(p o) -> p o", o=1)
            )
            eq = spool.tile([P, D], F32, tag="ring_eq_sig")
            nc.gpsimd.indirect_dma_start(
                out=eq[:],
                out_offset=None,
                in_=teq_d.ap(),
                in_offset=bass.IndirectOffsetOnAxis(ap=offq[:], axis=0),
            )
            offk = spool.tile([P, 1], I32, tag="offk")
            nc.sync.dma_start(
                offk[:], xtk_d.ap()[ti * P : (ti + 1) * P].rearrange("(p o) -> p o", o=1)
            )
            ek = spool.tile([P, D], F32, tag="ek_ring")
            nc.gpsimd.indirect_dma_start(
                out=ek[:],
                out_offset=None,
                in_=tek_d.ap(),
                in_offset=bass.IndirectOffsetOnAxis(ap=offk[:], axis=0),
            )
            pts = {}
            for which in ("q", "k", "v"):
                pts[which] = psum.tile([P, CW], F32, tag="mm_ps", name=f"qkv_{which}")
            for kk in range(DK):
                for wi, (which, base) in enumerate(
                    (("q", 0), ("k", D), ("v", 2 * D))
                ):
                    nc.tensor.matmul(
                        pts[which][:, :D],
                        lhsT=hT[:, kk, ti * P : (ti + 1) * P],
                        rhs=war[:, kk, base : base + D],
                        start=(kk == 0),
                        stop=(kk == DK - 1),
                    )
            nc.vector.tensor_tensor(q_sb[:, ti, :], pts["q"][:, :D], eq[:], ALU.add)
            nc.vector.tensor_tensor(k_sb[:, ti, :], pts["k"][:, :D], ek[:], ALU.add)
            nc.scalar.copy(
                vext[:, ti, :, 0:HD],
                pts["v"][:, :D].rearrange("p (h x) -> p h x", h=H),
            )

        # ---------------- RoPE (token-major, in place) --------------------
        # Build sin/cos tables for all 8 token tiles at once: [P, TT, 32]
        def rope_tables(pos_sb, tagp):
            fr = wpool.tile([P, TT, 16], F32, tag="rp_fr", name=f"fr_{tagp}")
            nc.vector.tensor_tensor(
                fr[:],
                pos_sb[:].unsqueeze(2).broadcast_to((P, TT, 16)),
                invf[:].unsqueeze(1).broadcast_to((P, TT, 16)),
                ALU.mult,
            )

            def lut_arg(tag, quarter):
                y = wpool.tile([P, TT, 16], F32, tag="rp_y", name=f"y_{tag}_{tagp}")
                nc.vector.tensor_scalar(
                    y[:], fr[:], INV_2PI, 0.25 if quarter else 0.0,
                    ALU.mult, ALU.add,
                )
                nc.vector.tensor_scalar(
                    y[:], y[:], MAGIC, MAGIC, ALU.add, ALU.subtract
                )
                nc.vector.scalar_tensor_tensor(
                    y[:], y[:], -TWO_PI, fr[:], ALU.mult, ALU.add
                )
                if quarter:
                    nc.vector.tensor_scalar_add(y[:], y[:], float(np.pi / 2))
                sc = wpool.tile([P, TT, 16], BF16, tag=f"rp_s{tag}{tagp}", name=f"sc_{tag}_{tagp}")
                nc.scalar.activation(sc[:], y[:], AF.Sin)
                return sc

            sin16 = lut_arg("s", False)
            cos16 = lut_arg("c", True)
            cos32 = wpool.tile([P, TT, 16, 2], BF16, tag=f"rp_cos32{tagp}")
            nc.vector.tensor_copy(cos32[:, :, :, 0], cos16[:])
            nc.vector.tensor_copy(cos32[:, :, :, 1], cos16[:])
            sin32 = wpool.tile([P, TT, 16, 2], BF16, tag=f"rp_sin32{tagp}")
            nc.scalar.mul(sin32[:, :, :, 0], sin16[:], -1.0)
            nc.vector.tensor_copy(sin32[:, :, :, 1], sin16[:])
            return cos32, sin32

        cosq, sinq = rope_tables(posq_sb, "q")
        cosk, sink = rope_tables(posk_sb, "k")

        def rope_tile(dst, ti, cos32, sin32):
            rot = (
                dst[:, ti, :]
                .rearrange("p (h x) -> p h x", h=H)[:, :, 0:DPR]
                .rearrange("p h (u v) -> p h u v", v=2)
            )
            shuf = _ap_with(rot, 1, [rot.ap[0], rot.ap[1], rot.ap[2], [-1, 2]])
            sin_b = sin32[:, ti].unsqueeze(1).broadcast_to((P, H, 16, 2))
            cos_b = cos32[:, ti].unsqueeze(1).broadcast_to((P, H, 16, 2))
            tmp = spool.tile([P, H, 16, 2], BF16, tag="rp_tmp", bufs=1)
            nc.vector.tensor_tensor(tmp[:], shuf, sin_b, ALU.mult)
            nc.vector.tensor_tensor(rot, rot, cos_b, ALU.mult)
            nc.vector.tensor_tensor(rot, rot, tmp[:], ALU.add)

        for ti in range(TT):
            rope_tile(q_sb, ti, cosq, sinq)
            rope_tile(k_sb, ti, cosk, sink)

        # ---------------- transpose q, k (qT reuses hT slot) --------------
        qT = work.tile([P, DK, T], BF16, tag="hT_qT")
        kT = work.tile([P, DK, T], BF16, tag="kT_gT")
        for ti in range(TT):
            for j in range(DK):
                transpose_128(
                    q_sb[:, ti, j * P : (j + 1) * P],
                    qT[:, j, ti * P : (ti + 1) * P],
                )
                transpose_128(
                    k_sb[:, ti, j * P : (j + 1) * P],
                    kT[:, j, ti * P : (ti + 1) * P],
                )

        # ---------------- attention ----------------
        x_new = work.tile([P, TT, D], F32, tag="x_new")
        for j in range(H // 2):
            expTs = []
            for sub in range(2):
                expTs.append(
                    work.tile([P, TT, CW], BF16, tag="expT_bufs", bufs=2,
                              name=f"expT_{j}_{sub}")
                )
            oTs = []
            for sub in range(2):
                oTs.append(
                    work.tile([NH, T], F32, tag="oT", bufs=2,
                              name=f"oT_{j}_{sub}")
                )
            for c in range(NCH):
                lim = 4 * c + 4
                for ti in range(lim):
                    pss = []
                    for sub in range(2):
                        r0 = 64 * sub
                        ps = psum.tile([P, CW], F32, tag="mm_ps",
                                       name=f"sc_{j}_{sub}")
                        # heads 2j (rows 0-63) and 2j+1 (rows 64-127) run
                        # concurrently on disjoint PE row groups
                        nc.tensor.matmul(
                            ps[:],
                            lhsT=kT[r0 : r0 + HD, j, ti * P : (ti + 1) * P],
                            rhs=qT[r0 : r0 + HD, j, c * CW : (c + 1) * CW],
                            start=True,
                            stop=True,
                        )
                        pss.append(ps)
                    off = P * (ti - 4 * c)
                    for sub in range(2):
                        expT = expTs[sub]
                        ps = pss[sub]
                        if off <= -P:
                            nc.scalar.activation(
                                expT[:, ti, :], ps[:], AF.Exp, scale=0.125
                            )
                        else:
                            nc.scalar.activation(
                                expT[:, ti, off:CW], ps[:, off:CW], AF.Exp,
                                scale=0.125,
                            )
                            if off > 0:
                                nc.gpsimd.memset(expT[:, ti, 0:off], 0.0)
                            nc.gpsimd.affine_select(
                                out=expT[:, ti, off : off + P],
                                in_=expT[:, ti, off : off + P],
                                pattern=[[1, P]],
                                compare_op=ALU.is_ge,
                                fill=0.0,
                                base=0,
                                channel_multiplier=-1,
                            )
                pos = []
                for sub in range(2):
                    pos.append(psum_o.tile([P, CW], F32, tag="o_ps",
                                           name=f"po_{j}_{sub}"))
                for ti in range(lim):
                    for sub in range(2):
                        nc.tensor.matmul(
                            pos[sub][0:NH, :],
                            lhsT=vext[:, ti, 2 * j + sub, :],
                            rhs=expTs[sub][:, ti, :],
                            start=(ti == 0),
                            stop=(ti == lim - 1),
                        )
                for sub in range(2):
                    nc.vector.tensor_copy(
                        oTs[sub][:, c * CW : (c + 1) * CW], pos[sub][0:NH, :]
                    )
            for ti in range(TT):
                for sub in range(2):
                    hh = 2 * j + sub
                    pt = psum_tr.tile([P, P], F32, tag="tr_ps",
                                      name=f"tro_{j}_{sub}")
                    nc.tensor.matmul(
                        pt[:, 0:NH],
                        lhsT=oTs[sub][:, ti * P : (ti + 1) * P],
                        rhs=ident[0:NH, 0:NH],
                        is_transpose=True,
                        start=True,
                        stop=True,
                    )
                    rec = spool.tile([P, 1], F32, tag="rec")
                    nc.vector.reciprocal(rec[:], pt[:, HD : HD + 1])
                    nc.vector.scalar_tensor_tensor(
                        x_new[:, ti, hh * HD : (hh + 1) * HD],
                        pt[:, 0:HD],
                        rec[:],
                        xs[:, ti, hh * HD : (hh + 1) * HD],
                        ALU.mult,
                        ALU.add,
                    )

        # ---------------- FFN weights (reuse attention weight slots) ------
        w1r = load_weight_f32r(w1_d.ap(), DK, 2 * DFF, tag="w_big", eng="vector")
        w2r = load_weight_f32r(w2_d.ap(), DFF // P, D, tag="vext_w2", eng="vector")

        # ---------------- LN2 + transpose h2 (h2T reuses xs slot) ---------
        h2T = work.tile([P, DK, T], F32R, tag="xs_h2T")
        for ti in range(TT):
            h2_t = spool.tile([P, D], F32R, tag="h_ring_f")
            layernorm_tile(x_new[:, ti, :], h2_t[:], g2_bc, b2_bc, trivial_ln2)
            for j in range(DK):
                transpose_128(
                    h2_t[:, j * P : (j + 1) * P],
                    h2T[:, j, ti * P : (ti + 1) * P],
                )

        # ---------------- FFN ----------------
        gT = work.tile([P, DFF // P, T], F32R, tag="kT_gT")
        for m in range(DFF // P):
            sg = spool.tile([P, CW], F32, tag="ring_eq_sig")
            for c in range(NCH):
                pa = psum.tile([P, CW], F32, tag="mm_ps")
                pg = psum.tile([P, CW], F32, tag="mm_ps")
                for kk in range(DK):
                    nc.tensor.matmul(
                        pa[:],
                        lhsT=w1r[:, kk, m * P : (m + 1) * P],
                        rhs=h2T[:, kk, c * CW : (c + 1) * CW],
                        start=(kk == 0),
                        stop=(kk == DK - 1),
                    )
                for kk in range(DK):
                    nc.tensor.matmul(
                        pg[:],
                        lhsT=w1r[:, kk, DFF + m * P : DFF + (m + 1) * P],
                        rhs=h2T[:, kk, c * CW : (c + 1) * CW],
                        start=(kk == 0),
                        stop=(kk == DK - 1),
                    )
                cs = slice(c * CW, (c + 1) * CW)
                if trivial_b1:
                    nc.scalar.activation(sg[:], pg[:], AF.Sigmoid)
                    nc.vector.tensor_tensor(sg[:], pg[:], sg[:], ALU.mult)
                    nc.vector.tensor_tensor(gT[:, m, cs], pa[:], sg[:], ALU.mult)
                else:
                    bgap = bf1_sb[:, (DFF // P) + m : (DFF // P) + m + 1]
                    nc.scalar.activation(sg[:], pg[:], AF.Sigmoid, bias=bgap)
                    nc.vector.scalar_tensor_tensor(
                        sg[:], pg[:], bgap, sg[:], ALU.add, ALU.mult
                    )
                    nc.vector.scalar_tensor_tensor(
                        gT[:, m, cs], pa[:], bf1_sb[:, m : m + 1], sg[:],
                        ALU.add, ALU.mult,
                    )

        yT = work.tile([P, DK, T], F32R, tag="q_sb")
        for m in range(DK):
            for c in range(NCH):
                py = psum.tile([P, CW], F32, tag="mm_ps")
                for kk in range(DFF // P):
                    nc.tensor.matmul(
                        py[:],
                        lhsT=w2r[:, kk, m * P : (m + 1) * P],
                        rhs=gT[:, kk, c * CW : (c + 1) * CW],
                        start=(kk == 0),
                        stop=(kk == DFF // P - 1),
                    )
                cs = slice(c * CW, (c + 1) * CW)
                if trivial_b2:
                    nc.scalar.copy(yT[:, m, cs], py[:])
                else:
                    nc.vector.tensor_scalar_add(yT[:, m, cs], py[:], bf2_sb[:, m : m + 1])

        # ---------------- final transpose + residual + store -------------
        for ti in range(TT):
            fin = spool.tile([P, D], F32, tag="h_ring_f")
            for j in range(DK):
                pt = psum_tr.tile([P, P], F32R, tag="tr_ps")
                nc.tensor.transpose(pt[:], yT[:, j, ti * P : (ti + 1) * P], identr[:])
                nc.vector.tensor_tensor(
                    fin[:, j * P : (j + 1) * P],
                    pt[:],
                    x_new[:, ti, j * P : (j + 1) * P],
                    ALU.add,
                )
            nc.sync.dma_start(out_d.ap()[ti * P : (ti + 1) * P, :], fin[:])

        for p in (psum_o, psum_tr, psum, spool, work, wpool):
            p.release()

    return nc


_CACHE = {}


def _get_nc(key):
    if key not in _CACHE:
        _CACHE[key] = build_nc(*key)
    return _CACHE[key]


def make_in_maps(x_type, x_value, seq_order, W_attn, type_emb, ln1_g, ln1_b,
                 ln2_g, ln2_b, W1, b1, W2, b2):
    ident = np.eye(P, dtype=np.float32)
    inv_freq = 1.0 / (THETA ** (np.arange(0, DPR, 2, dtype=np.float32) / DPR))
    invf = np.tile(inv_freq[None, :], (P, 1)).astype(np.float32)
    in_maps = []
    for b in range(B):
        in_maps.append({
            "xv": np.ascontiguousarray(x_value[b], dtype=np.float32),
            "wa": np.asarray(W_attn, dtype=np.float32),
            "w1": np.asarray(W1, dtype=np.float32),
            "w2": np.asarray(W2, dtype=np.float32),
            "teq": np.ascontiguousarray(type_emb[:, :D], dtype=np.float32),
            "tek": np.ascontiguousarray(type_emb[:, D:], dtype=np.float32),
            "xtq": np.ascontiguousarray(x_type[b, :T], dtype=np.int32),
            "xtk": np.ascontiguousarray(x_type[b, 1 : T + 1], dtype=np.int32),
            "posq": np.ascontiguousarray(seq_order[b, :T], dtype=np.float32),
            "posk": np.ascontiguousarray(seq_order[b, 1 : T + 1], dtype=np.float32),
            "ident": ident,
            "invf": invf,
            "g1": np.asarray(ln1_g, dtype=np.float32),
            "b1ln": np.asarray(ln1_b, dtype=np.float32),
            "g2": np.asarray(ln2_g, dtype=np.float32),
            "b2ln": np.asarray(ln2_b, dtype=np.float32),
            "bf1": np.asarray(b1, dtype=np.float32),
            "bf2": np.asarray(b2, dtype=np.float32),
        })
    return in_maps


def triviality_key(ln1_g, ln1_b, ln2_g, ln2_b, b1, b2):
    return (
        bool(np.all(np.asarray(ln1_g) == 1.0) and np.all(np.asarray(ln1_b) == 0.0)),
        bool(np.all(np.asarray(ln2_g) == 1.0) and np.all(np.asarray(ln2_b) == 0.0)),
        bool(np.all(np.asarray(b1) == 0.0)),
        bool(np.all(np.asarray(b2) == 0.0)),
    )


def kernel(x_type, x_value, seq_order, W_attn, type_emb, ln1_g, ln1_b,
           ln2_g, ln2_b, W1, b1, W2, b2, _trace=False):
    from concourse.bass_utils import run_bass_kernel_spmd

    key = triviality_key(ln1_g, ln1_b, ln2_g, ln2_b, b1, b2)
    nc = _get_nc(key)
    in_maps = make_in_maps(
        x_type, x_value, seq_order, W_attn, type_emb, ln1_g, ln1_b,
        ln2_g, ln2_b, W1, b1, W2, b2,
    )
    res = run_bass_kernel_spmd(nc, in_maps, list(range(B)), trace=_trace)
    out = np.stack([res.results[i]["out"] for i in range(B)], axis=0)
    kernel.last_results = res
    return out


# revision 29
# speedup vs baseline: 1.4137x; 1.0401x over previous
"""Trainium2 Bass kernel for nn_ChemROAR (single transformer block, B=8).

Sharding: data-parallel over batch — core b computes batch element b.
No collectives. Matmuls run in float32r (rounded fp32, 1 cycle/row on PE
vs 4 for plain fp32).

Self-contained: only imports from /opt/trn_rl_repo (present on the target
machine image); no sibling files.
"""
import sys
import types

sys.path.insert(0, "/opt/trn_rl_repo")

import numpy as np

import concourse.bass as bass
import concourse.mybir as mybir
import concourse.tile as tile
import concourse.tile_utils as tile_utils
from concourse.vector_clock import ScopedClock

F32 = mybir.dt.float32
F32R = mybir.dt.float32r
BF16 = mybir.dt.bfloat16
I32 = mybir.dt.int32
AF = mybir.ActivationFunctionType
ALU = mybir.AluOpType

P = 128
B, T, D, H, DFF, NTYPE = 8, 1024, 512, 8, 1024, 341
HD = D // H          # 64
DPR = 32             # rotary dims per head
TT = T // P          # 8 token tiles
DK = D // P          # 4 d chunks
EPS = 1e-5
THETA = 10000.0
TWO_PI = 6.283185307179586
INV_2PI = 1.0 / TWO_PI
MAGIC = 12582912.0   # 1.5 * 2**23 — round-to-nearest magic for fp32
NH = HD + 1          # v columns + ones column (softmax denominator)
NCH = 2              # Tq chunks per head
CW = T // NCH        # 512

# SBUF cap: tile_utils default (192 KiB/partition) is stale; cayman has
# 208 KiB usable. Stay a bit under.
tile_utils.max_sbuf_usage = 207 * 1024

# ---------------------------------------------------------------------------
# Patch 1: the public walrus accepts only ONE attached sync-wait per
# instruction. Split excess waits onto standalone NoOps placed before the
# instruction (and split the kernel-tail drain into a chain of drains).
# ---------------------------------------------------------------------------
_MAXW = 1


def _install_tile_patch():
    if getattr(tile.TileContext, "_chemroar_patched", False):
        return
    orig_commit = tile.TileContext._commit_instruction

    def _commit_instruction(self, inst, lazy_reg_writes=True):
        si = getattr(inst, "sync_info", None)
        if si is not None and si.on_wait:
            waits = list(si.on_wait)
            if len(waits) > _MAXW:
                keep = waits[:_MAXW]
                excess = waits[_MAXW:]
                for i in range(0, len(excess), _MAXW):
                    nop = mybir.InstNoOp(
                        name=self.nc.get_next_instruction_name(),
                        ins=[],
                        outs=[],
                        sync_info=mybir.SyncInfo(
                            on_wait=excess[i : i + _MAXW], on_update=[]
                        ),
                        bass_nofuse=True,
                        engine=inst.engine,
                    )
                    self._add_instruction(nop)
                inst.sync_info = mybir.SyncInfo(
                    on_wait=keep, on_update=list(si.on_update)
                )
        return orig_commit(self, inst, lazy_reg_writes=lazy_reg_writes)

    def _drain_and_barrier(self, tick_clock, wait_clock):
        drain_inst = self.nc.sync.drain()
        wait_clock.add_sem_waits(
            drain_inst.ins, ScopedClock({None: tick_clock.global_clock})
        )
        mi = drain_inst.ins
        si = mi.sync_info
        if si is not None and si.on_wait and len(si.on_wait) > _MAXW:
            waits = list(si.on_wait)
            mi.sync_info = mybir.SyncInfo(
                on_wait=waits[:_MAXW], on_update=list(si.on_update)
            )
            for i in range(_MAXW, len(waits), _MAXW):
                d2 = self.nc.sync.drain()
                d2.ins.sync_info = mybir.SyncInfo(
                    on_wait=waits[i : i + _MAXW], on_update=[]
                )
        self.nc.all_engine_barrier()
        assert self.sems is not None
        popped = self.nc._tile_sem_poison_stack.pop()
        assert popped is self._sem_poison
        self.nc.clear_and_free_semaphores(list(self.sems.allocated().values()))
        self.nc.all_engine_barrier()

    tile.TileContext._commit_instruction = _commit_instruction
    tile.TileContext._drain_and_barrier = _drain_and_barrier
    tile.TileContext._chemroar_patched = True


_install_tile_patch()


# ---------------------------------------------------------------------------
# Patch 2: NTFF profile hook (the stripped antenv lacks axon_hooks).
# ---------------------------------------------------------------------------
def _install_hookfix():
    name = "antenv.axon_hooks"
    if name in sys.modules:
        return
    try:
        from trn_agent_boot.trn_boot import _ntff_profile_via_ctypes

        hook = _ntff_profile_via_ctypes("/opt/axon/libaxon_pjrt.so")
    except Exception:
        hook = None
    mod = types.ModuleType(name)
    mod._hook = hook
    mod.set_axon_ntff_profile_hook = lambda h: setattr(mod, "_hook", h)
    mod.get_axon_ntff_profile_hook = lambda: mod._hook
    sys.modules[name] = mod
    try:
        import antenv

        antenv.axon_hooks = mod
    except Exception:
        pass


_install_hookfix()


def _ap_with(a, offset_delta, ap_list):
    import dataclasses

    return dataclasses.replace(a, offset=a.offset + offset_delta, ap=ap_list)


def build_nc(trivial_ln1, trivial_ln2, trivial_b1, trivial_b2):
    nc = bass.Bass("TRN2", target_bir_lowering=False, debug=False)

    xv_d = nc.declare_dram_parameter("xv", [T, D], F32, isOutput=False)
    wa_d = nc.declare_dram_parameter("wa", [D, 3 * D], F32, isOutput=False)
    w1_d = nc.declare_dram_parameter("w1", [D, 2 * DFF], F32, isOutput=False)
    w2_d = nc.declare_dram_parameter("w2", [DFF, D], F32, isOutput=False)
    teq_d = nc.declare_dram_parameter("teq", [NTYPE, D], F32, isOutput=False)
    tek_d = nc.declare_dram_parameter("tek", [NTYPE, D], F32, isOutput=False)
    xtq_d = nc.declare_dram_parameter("xtq", [T], I32, isOutput=False)
    xtk_d = nc.declare_dram_parameter("xtk", [T], I32, isOutput=False)
    posq_d = nc.declare_dram_parameter("posq", [T], F32, isOutput=False)
    posk_d = nc.declare_dram_parameter("posk", [T], F32, isOutput=False)
    ident_d = nc.declare_dram_parameter("ident", [P, P], F32, isOutput=False)
    invf_d = nc.declare_dram_parameter("invf", [P, 16], F32, isOutput=False)
    g1_d = nc.declare_dram_parameter("g1", [D], F32, isOutput=False)
    b1ln_d = nc.declare_dram_parameter("b1ln", [D], F32, isOutput=False)
    g2_d = nc.declare_dram_parameter("g2", [D], F32, isOutput=False)
    b2ln_d = nc.declare_dram_parameter("b2ln", [D], F32, isOutput=False)
    bf1_d = nc.declare_dram_parameter("bf1", [2 * DFF], F32, isOutput=False)
    bf2_d = nc.declare_dram_parameter("bf2", [D], F32, isOutput=False)
    out_d = nc.declare_dram_parameter("out", [T, D], F32, isOutput=True)

    with tile.TileContext(nc) as tc:
        wpool = tc.alloc_tile_pool(name="wpool", bufs=1)
        work = tc.alloc_tile_pool(name="work", bufs=1)
        spool = tc.alloc_tile_pool(name="spool", bufs=2)
        psum = tc.alloc_tile_pool(name="psum", bufs=4, space="PSUM")
        psum_tr = tc.alloc_tile_pool(name="psum_tr", bufs=2, space="PSUM")
        psum_o = tc.alloc_tile_pool(name="psum_o", bufs=2, space="PSUM")

        # ---------------- small constants ----------------
        ident = wpool.tile([P, P], F32)
        nc.sync.dma_start(ident[:], ident_d.ap())
        identr = wpool.tile([P, P], F32R)
        nc.gpsimd.tensor_copy(identr[:], ident[:])

        invf = wpool.tile([P, 16], F32)
        nc.sync.dma_start(invf[:], invf_d.ap())

        posq_sb = wpool.tile([P, TT], F32)
        nc.sync.dma_start(posq_sb[:], posq_d.ap().rearrange("(a p) -> p a", p=P))
        posk_sb = wpool.tile([P, TT], F32)
        nc.sync.dma_start(posk_sb[:], posk_d.ap().rearrange("(a p) -> p a", p=P))

        if not trivial_b1:
            bf1_sb = wpool.tile([P, 2 * DFF // P], F32)
            nc.sync.dma_start(bf1_sb[:], bf1_d.ap().rearrange("(o p) -> p o", p=P))
        if not trivial_b2:
            bf2_sb = wpool.tile([P, DK], F32)
            nc.sync.dma_start(bf2_sb[:], bf2_d.ap().rearrange("(o p) -> p o", p=P))

        def load_weight_f32r(dram_ap, ko, n, tag, eng="gpsimd"):
            """DMA [ko*P, n] DRAM weight, cast to BF16 via a scratch ring."""
            wr = work.tile([P, ko, n], BF16, tag=tag)
            src = dram_ap.rearrange("(ko ki) n -> ki ko n", ki=P)
            CHW = 512
            for k in range(ko):
                for c0 in range(0, n, CHW):
                    w = min(CHW, n - c0)
                    sc = spool.tile([P, CHW], F32, tag="wscratch", bufs=2)
                    nc.sync.dma_start(sc[:, :w], src[:, k, c0 : c0 + w])
                    if eng == "gpsimd":
                        nc.gpsimd.tensor_copy(wr[:, k, c0 : c0 + w], sc[:, :w])
                    else:
                        nc.vector.tensor_copy(wr[:, k, c0 : c0 + w], sc[:, :w])
            return wr

        # gamma/beta partition-broadcast tiles via K=1 matmul
        def bcast_row(src_dram, n, tag):
            row = wpool.tile([1, n], F32, tag=f"bcrow_{tag}")
            nc.sync.dma_start(row[:], src_dram.ap().rearrange("(o n) -> o n", o=1))
            rowr = wpool.tile([1, n], F32R, tag=f"bcrowr_{tag}")
            nc.vector.tensor_copy(rowr[:], row[:])
            onesc = wpool.tile([1, P], F32R, tag="bc_ones")
            nc.vector.memset(onesc[:], 1.0)
            out_t = wpool.tile([P, n], F32, tag=f"bcout_{tag}")
            for c0 in range(0, n, 512):
                w = min(512, n - c0)
                pt = psum_o.tile([P, CW], F32, tag="o_ps")
                nc.tensor.matmul(
                    pt[:, :w], lhsT=onesc[:], rhs=rowr[:, c0 : c0 + w],
                    start=True, stop=True,
                )
                nc.scalar.copy(out_t[:, c0 : c0 + w], pt[:, :w])
            return out_t

        g1_bc = b1_bc = g2_bc = b2_bc = None
        if not trivial_ln1:
            g1_bc = bcast_row(g1_d, D, "g1")
            b1_bc = bcast_row(b1ln_d, D, "b1")
        if not trivial_ln2:
            g2_bc = bcast_row(g2_d, D, "g2")
            b2_bc = bcast_row(b2ln_d, D, "b2")

        # attention weights now; FFN weights later (lifetime-shared tags)
        war = load_weight_f32r(wa_d.ap(), DK, 3 * D, tag="w_big")

        # ---------------- load x ----------------
        xs = work.tile([P, TT, D], F32, tag="xs_h2T")
        for ti in range(TT):
            nc.sync.dma_start(xs[:, ti, :], xv_d.ap()[ti * P : (ti + 1) * P, :])

        # ---------------- helpers ----------------
        def layernorm_tile(x_ap, out_ap, g_bc, b_bc, trivial):
            m = spool.tile([P, 1], F32, tag="ln_m")
            nc.vector.reduce_sum(m[:], x_ap, axis=mybir.AxisListType.X)
            nc.vector.tensor_scalar_mul(m[:], m[:], 1.0 / D)
            sq = spool.tile([P, 1], F32, tag="ln_sq")
            # out_ap doubles as junk output for the squared pass
            nc.scalar.activation(out_ap, x_ap, AF.Square, accum_out=sq[:])
            mm2 = spool.tile([P, 1], F32, tag="ln_mm2")
            nc.vector.tensor_tensor(mm2[:], m[:], m[:], ALU.mult)
            s = spool.tile([P, 1], F32, tag="ln_s")
            nc.vector.tensor_scalar(s[:], sq[:], 1.0 / D, EPS, ALU.mult, ALU.add)
            nc.vector.tensor_tensor(s[:], s[:], mm2[:], ALU.subtract)
            nc.vector.reciprocal(s[:], s[:])
            nc.scalar.sqrt(s[:], s[:])
            if trivial:
                nc.vector.tensor_scalar(out_ap, x_ap, m[:], s[:],
                                        ALU.subtract, ALU.mult)
            else:
                tmp = spool.tile([P, D], F32, tag="ring_eq_sig")
                nc.vector.tensor_scalar(tmp[:], x_ap, m[:], s[:],
                                        ALU.subtract, ALU.mult)
                nc.vector.tensor_tensor(tmp[:], tmp[:], g_bc[:], ALU.mult)
                nc.vector.tensor_tensor(out_ap, tmp[:], b_bc[:], ALU.add)

        _tr_flip = [0]

        def transpose_128(src_ap, dst_ap):
            pt = psum_tr.tile([P, P], F32R, tag="tr_ps")
            nc.tensor.transpose(pt[:], src_ap, identr[:])
            _tr_flip[0] ^= 1
            if _tr_flip[0]:
                nc.vector.tensor_copy(dst_ap, pt[:])
            else:
                nc.scalar.copy(dst_ap, pt[:])

        # ---------------- LN1 + transpose h (per-tile ring) --------------
        hT = work.tile([P, DK, T], BF16, tag="hT_qT")
        for ti in range(TT):
            h_t = spool.tile([P, D], F32R, tag="h_ring_f")
            layernorm_tile(xs[:, ti, :], h_t[:], g1_bc, b1_bc, trivial_ln1)
            for j in range(DK):
                transpose_128(
                    h_t[:, j * P : (j + 1) * P],
                    hT[:, j, ti * P : (ti + 1) * P],
                )

        # ---------------- qkv + emb + rope + transpose --------------------
        # NOTE: qT shares the hT tag slot, so allocate it only after hT's
        # last use. We therefore first compute q_sb/k_sb/vext fully.
        q_sb = work.tile([P, TT, D], F32R, tag="q_sb")
        k_sb = work.tile([P, TT, D], F32R, tag="k_sb")
        vext = work.tile([P, TT, H, NH], BF16, tag="vext_w2")
        onesf = wpool.tile([P, H], F32, tag="onesf")
        nc.gpsimd.memset(onesf[:], 1.0)
        for ti in range(TT):
            nc.gpsimd.tensor_copy(
                vext[:, ti, :, HD : HD + 1],
                onesf[:].rearrange("p (h o) -> p h o", o=1),
            )

        for ti in range(TT):
            # emb gathers for this tile
            offq = spool.tile([P, 1], I32, tag="offq")
            nc.sync.dma_start(
                offq[:], xtq_d.ap()[ti * P : (ti + 1) * P].rearrange("(p o) -> p o", o=1)
            )
            eq = spool.tile([P, D], F32, tag="ring_eq_sig")
            nc.gpsimd.indirect_dma_start(
                out=eq[:],
                out_offset=None,
                in_=teq_d.ap(),
                in_offset=bass.IndirectOffsetOnAxis(ap=offq[:], axis=0),
            )
            offk = spool.tile([P, 1], I32, tag="offk")
            nc.sync.dma_start(
                offk[:], xtk_d.ap()[ti * P : (ti + 1) * P].rearrange("(p o) -> p o", o=1)
            )
            ek = spool.tile([P, D], F32, tag="ek_ring")
            nc.gpsimd.indirect_dma_start(
                out=ek[:],
                out_offset=None,
                in_=tek_d.ap(),
                in_offset=bass.IndirectOffsetOnAxis(ap=offk[:], axis=0),
            )
            pts = {}
            for which in ("q", "k", "v"):
                pts[which] = psum.tile([P, CW], F32, tag="mm_ps", name=f"qkv_{which}")
            for kk in range(DK):
                for wi, (which, base) in enumerate(
                    (("q", 0), ("k", D), ("v", 2 * D))
                ):
                    nc.tensor.matmul(
                        pts[which][:, :D],
                        lhsT=hT[:, kk, ti * P : (ti + 1) * P],
                        rhs=war[:, kk, base : base + D],
                        start=(kk == 0),
                        stop=(kk == DK - 1),
                    )
            nc.vector.tensor_tensor(q_sb[:, ti, :], pts["q"][:, :D], eq[:], ALU.add)
            nc.vector.tensor_tensor(k_sb[:, ti, :], pts["k"][:, :D], ek[:], ALU.add)
            nc.scalar.copy(
                vext[:, ti, :, 0:HD],
                pts["v"][:, :D].rearrange("p (h x) -> p h x", h=H),
            )

        # ---------------- RoPE (token-major, in place) --------------------
        # Build sin/cos tables for all 8 token tiles at once: [P, TT, 32]
        def rope_tables(pos_sb, tagp):
            fr = wpool.tile([P, TT, 16], F32, tag="rp_fr", name=f"fr_{tagp}")
            nc.vector.tensor_tensor(
                fr[:],
                pos_sb[:].unsqueeze(2).broadcast_to((P, TT, 16)),
                invf[:].unsqueeze(1).broadcast_to((P, TT, 16)),
                ALU.mult,
            )

            def lut_arg(tag, quarter):
                y = wpool.tile([P, TT, 16], F32, tag="rp_y", name=f"y_{tag}_{tagp}")
                nc.vector.tensor_scalar(
                    y[:], fr[:], INV_2PI, 0.25 if quarter else 0.0,
                    ALU.mult, ALU.add,
                )
                nc.vector.tensor_scalar(
                    y[:], y[:], MAGIC, MAGIC, ALU.add, ALU.subtract
                )
                nc.vector.scalar_tensor_tensor(
                    y[:], y[:], -TWO_PI, fr[:], ALU.mult, ALU.add
                )
                if quarter:
                    nc.vector.tensor_scalar_add(y[:], y[:], float(np.pi / 2))
                sc = wpool.tile([P, TT, 16], BF16, tag=f"rp_s{tag}{tagp}", name=f"sc_{tag}_{tagp}")
                nc.scalar.activation(sc[:], y[:], AF.Sin)
                return sc

            sin16 = lut_arg("s", False)
            cos16 = lut_arg("c", True)
            cos32 = wpool.tile([P, TT, 16, 2], BF16, tag=f"rp_cos32{tagp}")
            nc.vector.tensor_copy(cos32[:, :, :, 0], cos16[:])
            nc.vector.tensor_copy(cos32[:, :, :, 1], cos16[:])
            sin32 = wpool.tile([P, TT, 16, 2], BF16, tag=f"rp_sin32{tagp}")
            nc.scalar.mul(sin32[:, :, :, 0], sin16[:], -1.0)
            nc.vector.tensor_copy(sin32[:, :, :, 1], sin16[:])
            return cos32, sin32

        cosq, sinq = rope_tables(posq_sb, "q")
        cosk, sink = rope_tables(posk_sb, "k")

        def rope_tile(dst, ti, cos32, sin32):
            rot = (
                dst[:, ti, :]
                .rearrange("p (h x) -> p h x", h=H)[:, :, 0:DPR]
                .rearrange("p h (u v) -> p h u v", v=2)
            )
            shuf = _ap_with(rot, 1, [rot.ap[0], rot.ap[1], rot.ap[2], [-1, 2]])
            sin_b = sin32[:, ti].unsqueeze(1).broadcast_to((P, H, 16, 2))
            cos_b = cos32[:, ti].unsqueeze(1).broadcast_to((P, H, 16, 2))
            tmp = spool.tile([P, H, 16, 2], BF16, tag="rp_tmp", bufs=1)
            nc.vector.tensor_tensor(tmp[:], shuf, sin_b, ALU.mult)
            nc.vector.tensor_tensor(rot, rot, cos_b, ALU.mult)
            nc.vector.tensor_tensor(rot, rot, tmp[:], ALU.add)

        for ti in range(TT):
            rope_tile(q_sb, ti, cosq, sinq)
            rope_tile(k_sb, ti, cosk, sink)

        # ---------------- transpose q, k (qT reuses hT slot) --------------
        qT = work.tile([P, DK, T], BF16, tag="hT_qT")
        kT = work.tile([P, DK, T], BF16, tag="kT_gT")
        for ti in range(TT):
            for j in range(DK):
                transpose_128(
                    q_sb[:, ti, j * P : (j + 1) * P],
                    qT[:, j, ti * P : (ti + 1) * P],
                )
                transpose_128(
                    k_sb[:, ti, j * P : (j + 1) * P],
                    kT[:, j, ti * P : (ti + 1) * P],
                )

        # ---------------- attention ----------------
        x_new = work.tile([P, TT, D], F32, tag="x_new")
        for j in range(H // 2):
            expTs = []
            for sub in range(2):
                expTs.append(
                    work.tile([P, TT, CW], BF16, tag="expT_bufs", bufs=2,
                              name=f"expT_{j}_{sub}")
                )
            oTs = []
            for sub in range(2):
                oTs.append(
                    work.tile([NH, T], F32, tag="oT", bufs=2,
                              name=f"oT_{j}_{sub}")
                )
            for c in range(NCH):
                lim = 4 * c + 4
                for ti in range(lim):
                    pss = []
                    for sub in range(2):
                        r0 = 64 * sub
                        ps = psum.tile([P, CW], F32, tag="mm_ps",
                                       name=f"sc_{j}_{sub}")
                        # heads 2j (rows 0-63) and 2j+1 (rows 64-127) run
                        # concurrently on disjoint PE row groups
                        nc.tensor.matmul(
                            ps[:],
                            lhsT=kT[r0 : r0 + HD, j, ti * P : (ti + 1) * P],
                            rhs=qT[r0 : r0 + HD, j, c * CW : (c + 1) * CW],
                            start=True,
                            stop=True,
                        )
                        pss.append(ps)
                    off = P * (ti - 4 * c)
                    for sub in range(2):
                        expT = expTs[sub]
                        ps = pss[sub]
                        if off <= -P:
                            nc.scalar.activation(
                                expT[:, ti, :], ps[:], AF.Exp, scale=0.125
                            )
                        else:
                            nc.scalar.activation(
                                expT[:, ti, off:CW], ps[:, off:CW], AF.Exp,
                                scale=0.125,
                            )
                            if off > 0:
                                nc.gpsimd.memset(expT[:, ti, 0:off], 0.0)
                            nc.gpsimd.affine_select(
                                out=expT[:, ti, off : off + P],
                                in_=expT[:, ti, off : off + P],
                                pattern=[[1, P]],
                                compare_op=ALU.is_ge,
                                fill=0.0,
                                base=0,
                                channel_multiplier=-1,
                            )
                pos = []
                for sub in range(2):
                    pos.append(psum_o.tile([P, CW], F32, tag="o_ps",
                                           name=f"po_{j}_{sub}"))
                for ti in range(lim):
                    for sub in range(2):
                        nc.tensor.matmul(
                            pos[sub][0:NH, :],
                            lhsT=vext[:, ti, 2 * j + sub, :],
                            rhs=expTs[sub][:, ti, :],
                            start=(ti == 0),
                            stop=(ti == lim - 1),
                        )
                for sub in range(2):
                    nc.vector.tensor_copy(
                        oTs[sub][:, c * CW : (c + 1) * CW], pos[sub][0:NH, :]
                    )
            for ti in range(TT):
                for sub in range(2):
                    hh = 2 * j + sub
                    pt = psum_tr.tile([P, P], F32, tag="tr_ps",
                                      name=f"tro_{j}_{sub}")
                    nc.tensor.matmul(
                        pt[:, 0:NH],
                        lhsT=oTs[sub][:, ti * P : (ti + 1) * P],
                        rhs=ident[0:NH, 0:NH],
                        is_transpose=True,
                        start=True,
                        stop=True,
                    )
                    rec = spool.tile([P, 1], F32, tag="rec")
                    nc.vector.reciprocal(rec[:], pt[:, HD : HD + 1])
                    nc.vector.scalar_tensor_tensor(
                        x_new[:, ti, hh * HD : (hh + 1) * HD],
                        pt[:, 0:HD],
                        rec[:],
                        xs[:, ti, hh * HD : (hh + 1) * HD],
                        ALU.mult,
                        ALU.add,
                    )

        # ---------------- FFN weights (reuse attention weight slots) ------
        w1r = load_weight_f32r(w1_d.ap(), DK, 2 * DFF, tag="w_big", eng="vector")
        w2r = load_weight_f32r(w2_d.ap(), DFF // P, D, tag="vext_w2", eng="vector")

        # ---------------- LN2 + transpose h2 (h2T reuses xs slot) ---------
        h2T = work.tile([P, DK, T], BF16, tag="xs_h2T")
        for ti in range(TT):
            h2_t = spool.tile([P, D], F32R, tag="h_ring_f")
            layernorm_tile(x_new[:, ti, :], h2_t[:], g2_bc, b2_bc, trivial_ln2)
            for j in range(DK):
                transpose_128(
                    h2_t[:, j * P : (j + 1) * P],
                    h2T[:, j, ti * P : (ti + 1) * P],
                )

        # ---------------- FFN ----------------
        gT = work.tile([P, DFF // P, T], BF16, tag="kT_gT")
        for m in range(DFF // P):
            sg = spool.tile([P, CW], F32, tag="ring_eq_sig")
            for c in range(NCH):
                pa = psum.tile([P, CW], F32, tag="mm_ps")
                pg = psum.tile([P, CW], F32, tag="mm_ps")
                for kk in range(DK):
                    nc.tensor.matmul(
                        pa[:],
                        lhsT=w1r[:, kk, m * P : (m + 1) * P],
                        rhs=h2T[:, kk, c * CW : (c + 1) * CW],
                        start=(kk == 0),
                        stop=(kk == DK - 1),
                    )
                for kk in range(DK):
                    nc.tensor.matmul(
                        pg[:],
                        lhsT=w1r[:, kk, DFF + m * P : DFF + (m + 1) * P],
                        rhs=h2T[:, kk, c * CW : (c + 1) * CW],
                        start=(kk == 0),
                        stop=(kk == DK - 1),
                    )
                cs = slice(c * CW, (c + 1) * CW)
                if trivial_b1:
                    nc.scalar.activation(sg[:], pg[:], AF.Sigmoid)
                    nc.vector.tensor_tensor(sg[:], pg[:], sg[:], ALU.mult)
                    nc.vector.tensor_tensor(gT[:, m, cs], pa[:], sg[:], ALU.mult)
                else:
                    bgap = bf1_sb[:, (DFF // P) + m : (DFF // P) + m + 1]
                    nc.scalar.activation(sg[:], pg[:], AF.Sigmoid, bias=bgap)
                    nc.vector.scalar_tensor_tensor(
                        sg[:], pg[:], bgap, sg[:], ALU.add, ALU.mult
                    )
                    nc.vector.scalar_tensor_tensor(
                        gT[:, m, cs], pa[:], bf1_sb[:, m : m + 1], sg[:],
                        ALU.add, ALU.mult,
                    )

        yT = work.tile([P, DK, T], F32R, tag="q_sb")
        for m in range(DK):
            for c in range(NCH):
                py = psum.tile([P, CW], F32, tag="mm_ps")
                for kk in range(DFF // P):
                    nc.tensor.matmul(
                        py[:],
                        lhsT=w2r[:, kk, m * P : (m + 1) * P],
                        rhs=gT[:, kk, c * CW : (c + 1) * CW],
                        start=(kk == 0),
                        stop=(kk == DFF // P - 1),
                    )
                cs = slice(c * CW, (c + 1) * CW)
                if trivial_b2:
                    nc.scalar.copy(yT[:, m, cs], py[:])
                else:
                    nc.vector.tensor_scalar_add(yT[:, m, cs], py[:], bf2_sb[:, m : m + 1])

        # ---------------- final transpose + residual + store -------------
        for ti in range(TT):
            fin = spool.tile([P, D], F32, tag="h_ring_f")
            for j in range(DK):
                pt = psum_tr.tile([P, P], F32R, tag="tr_ps")
                nc.tensor.transpose(pt[:], yT[:, j, ti * P : (ti + 1) * P], identr[:])
                nc.vector.tensor_tensor(
                    fin[:, j * P : (j + 1) * P],
                    pt[:],
                    x_new[:, ti, j * P : (j + 1) * P],
                    ALU.add,
                )
            nc.sync.dma_start(out_d.ap()[ti * P : (ti + 1) * P, :], fin[:])

        for p in (psum_o, psum_tr, psum, spool, work, wpool):
            p.release()

    return nc


_CACHE = {}


def _get_nc(key):
    if key not in _CACHE:
        _CACHE[key] = build_nc(*key)
    return _CACHE[key]


def make_in_maps(x_type, x_value, seq_order, W_attn, type_emb, ln1_g, ln1_b,
                 ln2_g, ln2_b, W1, b1, W2, b2):
    ident = np.eye(P, dtype=np.float32)
    inv_freq = 1.0 / (THETA ** (np.arange(0, DPR, 2, dtype=np.float32) / DPR))
    invf = np.tile(inv_freq[None, :], (P, 1)).astype(np.float32)
    in_maps = []
    for b in range(B):
        in_maps.append({
            "xv": np.ascontiguousarray(x_value[b], dtype=np.float32),
            "wa": np.asarray(W_attn, dtype=np.float32),
            "w1": np.asarray(W1, dtype=np.float32),
            "w2": np.asarray(W2, dtype=np.float32),
            "teq": np.ascontiguousarray(type_emb[:, :D], dtype=np.float32),
            "tek": np.ascontiguousarray(type_emb[:, D:], dtype=np.float32),
            "xtq": np.ascontiguousarray(x_type[b, :T], dtype=np.int32),
            "xtk": np.ascontiguousarray(x_type[b, 1 : T + 1], dtype=np.int32),
            "posq": np.ascontiguousarray(seq_order[b, :T], dtype=np.float32),
            "posk": np.ascontiguousarray(seq_order[b, 1 : T + 1], dtype=np.float32),
            "ident": ident,
            "invf": invf,
            "g1": np.asarray(ln1_g, dtype=np.float32),
            "b1ln": np.asarray(ln1_b, dtype=np.float32),
            "g2": np.asarray(ln2_g, dtype=np.float32),
            "b2ln": np.asarray(ln2_b, dtype=np.float32),
            "bf1": np.asarray(b1, dtype=np.float32),
            "bf2": np.asarray(b2, dtype=np.float32),
        })
    return in_maps


def triviality_key(ln1_g, ln1_b, ln2_g, ln2_b, b1, b2):
    return (
        bool(np.all(np.asarray(ln1_g) == 1.0) and np.all(np.asarray(ln1_b) == 0.0)),
        bool(np.all(np.asarray(ln2_g) == 1.0) and np.all(np.asarray(ln2_b) == 0.0)),
        bool(np.all(np.asarray(b1) == 0.0)),
        bool(np.all(np.asarray(b2) == 0.0)),
    )


def kernel(x_type, x_value, seq_order, W_attn, type_emb, ln1_g, ln1_b,
           ln2_g, ln2_b, W1, b1, W2, b2, _trace=False):
    from concourse.bass_utils import run_bass_kernel_spmd

    key = triviality_key(ln1_g, ln1_b, ln2_g, ln2_b, b1, b2)
    nc = _get_nc(key)
    in_maps = make_in_maps(
        x_type, x_value, seq_order, W_attn, type_emb, ln1_g, ln1_b,
        ln2_g, ln2_b, W1, b1, W2, b2,
    )
    res = run_bass_kernel_spmd(nc, in_maps, list(range(B)), trace=_trace)
    out = np.stack([res.results[i]["out"] for i in range(B)], axis=0)
    kernel.last_results = res
    return out
